# revision 2
# baseline (speedup 1.0000x reference)
"""Trainium2 Bass kernel v2 for BaseDependentAttentionLayer (GNN message passing).

Design vs v1 baseline (739us cost-model):
  - Chunk-outer pipeline: kv-table chunk c+1 builds (PE/DMA) while chunk c's
    edges process (DVE/Pool/Act).  Removes the 175us serial phase-1 stall.
  - No q gather: per tile, PE transposes the sel one-hot (slot x origin),
    Act copies selT to SBUF, PE matmul broadcasts the 128-node q block to
    slots.  Kills 250k gather descs of DMA + Pool desc-gen.
  - Per-block accumulators live in SBUF f32 across all 4 chunks; single
    final epilogue (one act-table switch, big-slab instructions).
  - Metadata packed per slab in one i16 stream (kvi | oid | ew-bf16).
  - Engine placement: sel/qk/tree/reduce/ctb on DVE; ws + psum flushes +
    phase1 copies on Pool; selT copies + exp on Act; DMA issues on SP.
"""

import sys

sys.path.insert(0, "/opt/trn_rl_repo")

import numpy as np
import ml_dtypes

import concourse.bass as bass
import concourse.bacc as bacc
import concourse.mybir as mybir
from concourse.tile import TileContext
from concourse.bass_utils import run_bass_kernel_spmd

N = 100000
E = 1600000
D = 64
H = 4
HD = 16
NCORES = 8
NOWN = 12500            # nodes owned per core
NBLK = 98               # 128-node blocks per core
NB = NBLK * 128         # 12544 padded own nodes
NT = 100352             # padded global table rows (= 4 * 25088)
CH = 4                  # dest chunks (int16 gather index limit)
CHROWS = NT // CH       # 25088
SLAB_T = 48             # edge tiles per processing slab
QB = 8                  # tiles per selT/q-broadcast batch
LN_EPS = 1e-5
PAD_OID = 200.0

F32 = mybir.dt.float32
BF16 = mybir.dt.bfloat16
I16 = mybir.dt.int16
BF16_NP = ml_dtypes.bfloat16


def _build_structure(origins, dests):
    """Chunk-major packed cell layout (no per-cell 128-rounding).

    Cells (chunk, block) are packed back-to-back; only chunk streams are
    padded to 128.  Tiles spanning cell boundaries are handled with
    partition-sliced matmuls ("pieces").  Structure is shared by cores:
    per-cell slot count = max over cores.
    """
    owner = origins // NOWN
    per_core_raw = []
    cnts = np.zeros((NCORES, CH * NBLK), np.int64)
    for c in range(NCORES):
        m = owner == c
        o = (origins[m] - c * NOWN).astype(np.int32)
        d = dests[m].astype(np.int32)
        eids = np.nonzero(m)[0]
        blk = o >> 7
        chunk = d // CHROWS
        cell = chunk * NBLK + blk          # chunk-major
        order = np.argsort(cell, kind="stable")
        o, d, eids = o[order], d[order], eids[order]
        cnt = np.bincount(cell[order], minlength=CH * NBLK)
        cnts[c] = cnt
        per_core_raw.append((o, d, eids, cnt))
    # PE matmul partition-base must be 0/32/64, so keep cells 128-aligned
    cell_slots = ((cnts.max(0) + 127) // 128) * 128

    cell_off = np.zeros(CH * NBLK, np.int64)
    pieces_by_tile = {}
    tile_total = 0
    chunk_ranges = []                      # (tile0, ntiles) per chunk
    for c in range(CH):
        s0 = tile_total * 128
        pos = s0
        for b in range(NBLK):
            cell = c * NBLK + b
            cell_off[cell] = pos
            n = int(cell_slots[cell])
            if n == 0:
                continue
            p0 = pos
            while p0 < pos + n:
                t = p0 // 128
                lo = p0 - t * 128
                hi = min(128, pos + n - t * 128)
                pieces_by_tile.setdefault(t, []).append(
                    (b, lo, hi, p0 == pos, t * 128 + hi == pos + n, cell))
                p0 = t * 128 + hi
            pos += n
        ntiles = (pos - s0 + 127) // 128
        chunk_ranges.append((tile_total, ntiles))
        tile_total += ntiles
    S_tiles = tile_total

    slabs = []
    meta_off = 0
    for c in range(CH):
        t0c, tc = chunk_ranges[c]
        s = t0c
        while s < t0c + tc:
            T = min(SLAB_T, t0c + tc - s)
            slabs.append({
                "chunk": c, "t0": s, "T": T, "meta_off": meta_off,
                "pieces": {t - s: pieces_by_tile.get(t, [])
                           for t in range(s, s + T)},
            })
            meta_off += 13 * T
            s += T
    struct = {
        "slabs": slabs,
        "S_tiles": S_tiles,
        "meta_cols": meta_off,
        "cell_slots": cell_slots,
        "cell_off": cell_off,
    }
    return struct, per_core_raw


def _per_core_arrays(struct, core_raw, edge_weights):
    """Packed per-slab metadata stream [128, meta_cols] int16."""
    o, d, eids, cnt = core_raw
    S_tiles = struct["S_tiles"]
    S = S_tiles * 128
    cell_off = struct["cell_off"]

    oid = np.full(S, PAD_OID, np.float32)
    kvi = np.zeros(S, np.int16)
    ew4 = np.zeros((S, H), np.float32)

    cell_edge_off = np.zeros(CH * NBLK + 1, np.int64)
    np.cumsum(cnt, out=cell_edge_off[1:])
    for cell in range(CH * NBLK):
        n = int(cnt[cell])
        if n == 0:
            continue
        e0 = cell_edge_off[cell]
        s0 = int(cell_off[cell])
        ch = cell // NBLK
        sl = slice(s0, s0 + n)
        el = slice(e0, e0 + n)
        oid[sl] = (o[el] & 127).astype(np.float32)
        kvi[sl] = (d[el] - ch * CHROWS).astype(np.int16)
        ew4[sl] = edge_weights[eids[el]] * (HD ** -0.5)

    # slot-major -> tile layouts
    oid_t = np.ascontiguousarray(oid.reshape(S_tiles, 128).T).astype(BF16_NP)
    ew_t = np.ascontiguousarray(
        ew4.reshape(S_tiles, 128, H).transpose(1, 0, 2)).astype(BF16_NP)

    def wrap(run_vals):
        w = run_vals.reshape(-1, 16).T          # [16, len/16]
        return np.tile(w, (8, 1))               # [128, len/16]

    meta = np.zeros((128, struct["meta_cols"]), np.int16)
    for sl in struct["slabs"]:
        t0, T, mo = sl["t0"], sl["T"], sl["meta_off"]
        meta[:, mo:mo + 8 * T] = wrap(kvi[t0 * 128:(t0 + T) * 128])
        meta[:, mo + 8 * T:mo + 9 * T] = oid_t[:, t0:t0 + T].view(np.int16)
        meta[:, mo + 9 * T:mo + 13 * T] = (
            ew_t[:, t0:t0 + T, :].reshape(128, T * H).view(np.int16))
    return {"meta": meta}


def _build_graph(struct):
    nc = bacc.Bacc()
    slabs = struct["slabs"]

    xT = nc.declare_dram_parameter("xT", [D + 1, NT], BF16, isOutput=False)
    xTo = nc.declare_dram_parameter("xTo", [D + 1, NB], BF16, isOutput=False)
    wkv = nc.declare_dram_parameter("wkv", [D + 1, 2 * D], BF16, isOutput=False)
    wq = nc.declare_dram_parameter("wq", [D + 1, D], BF16, isOutput=False)
    wot = nc.declare_dram_parameter("wot", [D, D], F32, isOutput=False)
    boc = nc.declare_dram_parameter("boc", [128, D], F32, isOutput=False)
    gam = nc.declare_dram_parameter("gam", [128, D], F32, isOutput=False)
    iot = nc.declare_dram_parameter("iot", [128, 128, SLAB_T], BF16,
                                    isOutput=False)
    idn = nc.declare_dram_parameter("idn", [128, 128], BF16, isOutput=False)
    idnf = nc.declare_dram_parameter("idnf", [128, 128], F32, isOutput=False)
    xpb = nc.declare_dram_parameter("xpb", [NB, D], BF16, isOutput=False)
    met = nc.declare_dram_parameter("met", [128, struct["meta_cols"]], I16,
                                    isOutput=False)
    out = nc.declare_dram_parameter("out", [NB, D], F32, isOutput=True)

    kv_tab = nc.dram_tensor("kv_tab", [NT, 2 * D], BF16)

    with TileContext(nc) as tc:
        with tc.tile_pool(name="const", bufs=1) as cp:
            wkv_t = cp.tile([D + 1, 2 * D], BF16)
            nc.sync.dma_start(out=wkv_t[:], in_=wkv[:])
            wq_t = cp.tile([D + 1, D], BF16)
            nc.sync.dma_start(out=wq_t[:], in_=wq[:])
            wot_f = cp.tile([D, D], F32)
            nc.sync.dma_start(out=wot_f[:], in_=wot[:])
            wot_t = cp.tile([D, D], BF16)
            nc.vector.tensor_copy(wot_t[:], wot_f[:])
            boc_t = cp.tile([128, D], F32)
            nc.sync.dma_start(out=boc_t[:], in_=boc[:])
            gam_t = cp.tile([128, D], F32)
            nc.sync.dma_start(out=gam_t[:], in_=gam[:])
            iot_t = cp.tile([128, 128, SLAB_T], BF16)
            nc.sync.dma_start(out=iot_t[:], in_=iot[:])
            idn_t = cp.tile([128, 128], BF16)
            nc.sync.dma_start(out=idn_t[:], in_=idn[:])
            idnf_t = cp.tile([128, 128], F32)
            nc.sync.dma_start(out=idnf_t[:], in_=idnf[:])

            # q blocks for all own nodes, SBUF-resident [128, NBLK, D] bf16
            q_all = cp.tile([128, NBLK, D], BF16)
            # per-block f32 accumulators [128, NBLK, D+H]
            acc = cp.tile([128, NBLK, D + H], F32)
            nc.gpsimd.memset(acc[:], 0.0)

            # ---- q blocks: lhsT = xTo block cols, rhs = wq -> [128, 64]
            with (
                tc.tile_pool(name="qbx", bufs=2) as qbx,
                tc.tile_pool(name="qbp", bufs=2, space="PSUM") as qbp,
            ):
                for bat in range((NBLK + 7) // 8):
                    b0 = bat * 8
                    nb = min(8, NBLK - b0)
                    xs = qbx.tile([D + 1, 8 * 128], BF16, tag="xq")
                    nc.sync.dma_start(
                        out=xs[:, 0:nb * 128],
                        in_=xTo[:, b0 * 128:(b0 + nb) * 128])
                    ps = qbp.tile([128, 8, D], F32, tag="qp")
                    for j in range(nb):
                        nc.tensor.matmul(
                            out=ps[:, j, :],
                            lhsT=xs[:, j * 128:(j + 1) * 128],
                            rhs=wq_t[:],
                            start=True, stop=True)
                    nc.vector.tensor_copy(
                        q_all[:, b0:b0 + nb, :], ps[:, 0:nb, :])

            with (
                tc.tile_pool(name="p1x", bufs=2) as p1x,
                tc.tile_pool(name="p1s", bufs=2) as p1s,
                tc.tile_pool(name="p1p", bufs=1, space="PSUM") as p1p,
            ):
                # ---- kv table chunk builder: groups of 16 blocks (2048 rows)
                def build_kv_group(c, g):
                    # chunk 0 builds serially at startup while Act/SP are
                    # otherwise idle: split load/write across both queues
                    ld = nc.sync
                    wr = nc.sync
                    r0 = c * CHROWS + g * 2048
                    nrow = min(2048, CHROWS - g * 2048)
                    nblk4 = (nrow + 511) // 512     # 4-block psum batches
                    xs = p1x.tile([D + 1, 2048], BF16, tag="xs")
                    ld.dma_start(
                        out=xs[:, 0:nrow], in_=xT[:, r0:r0 + nrow])
                    sb = p1s.tile([128, 16, 2 * D], BF16, tag="sb")
                    for bat in range(nblk4):
                        ps = p1p.tile([128, 4, 2 * D], F32, tag="kp")
                        for j in range(4):
                            nc.tensor.matmul(
                                out=ps[:, j, :],
                                lhsT=xs[:, bat * 512 + j * 128:
                                        bat * 512 + (j + 1) * 128],
                                rhs=wkv_t[:],
                                start=True, stop=True)
                        nc.scalar.copy(
                            sb[:, bat * 4:(bat + 1) * 4, :]
                                .rearrange("p a d -> p (a d)"),
                            ps[:].rearrange("p a d -> p (a d)"))
                    wr.dma_start(
                        out=kv_tab[r0:r0 + nrow, :]
                            .rearrange("(a p) d -> p a d", p=128),
                        in_=sb[:, 0:nrow // 128, :])

                # ---- edge-processing slab
                with (
                    tc.tile_pool(name="gat", bufs=3) as gp,
                    tc.tile_pool(name="mp", bufs=3) as mp,
                    tc.tile_pool(name="sp", bufs=2) as sp,
                    tc.tile_pool(name="qs", bufs=3) as qp,
                    tc.tile_pool(name="wk", bufs=2) as wp,
                    tc.tile_pool(name="stp", bufs=3, space="PSUM") as stp,
                    tc.tile_pool(name="qpp", bufs=2, space="PSUM") as qpp,
                    tc.tile_pool(name="bpp", bufs=2, space="PSUM") as bpp,
                ):
                    state = {}
                    live = {}

                    def slab_A(sl):
                        c, t0, T = sl["chunk"], sl["t0"], sl["T"]
                        mo = sl["meta_off"]
                        mt = mp.tile([128, 13 * SLAB_T], I16, tag="mt")
                        nc.sync.dma_start(
                            out=mt[:, 0:13 * T], in_=met[:, mo:mo + 13 * T])
                        kvi_v = mt[:, 0:8 * T]
                        kvg = gp.tile([128, SLAB_T, 2 * D], BF16, tag="kvg")
                        nc.gpsimd.dma_gather(
                            out_ap=kvg[:, 0:T, :],
                            in_ap=kv_tab[c * CHROWS:(c + 1) * CHROWS, :],
                            idxs_ap=kvi_v,
                            num_idxs=T * 128,
                            num_idxs_reg=T * 128,
                            elem_size=2 * D,
                            single_packet=False)
                        state[sl["t0"]] = (mt, kvg)

                    def slab_B(sl):
                        c, t0, T = sl["chunk"], sl["t0"], sl["T"]
                        mt, kvg = state[t0]
                        oid_v = mt[:, 8 * T:9 * T].bitcast(BF16)

                        sel = sp.tile([128, 128, SLAB_T], BF16, tag="sel")
                        nc.vector.tensor_tensor(
                            out=sel[:, :, 0:T],
                            in0=oid_v.rearrange("p (o t) -> p o t", o=1)
                                .to_broadcast([128, 128, T]),
                            in1=iot_t[:, :, 0:T],
                            op=mybir.AluOpType.is_equal)

                        # q broadcast: transpose sel -> selT, q = selT^T @ Q,
                        # qk = q*k straight from PSUM (gpsimd can't read PSUM)
                        qk = qp.tile([128, SLAB_T, D], BF16, tag="qk")
                        nbat = (T + QB - 1) // QB
                        for b in range(nbat):
                            i0 = b * QB
                            nb_ = min(QB, T - i0)
                            tp = stp.tile([128, QB, 128], BF16, tag="tp")
                            for j in range(nb_):
                                nc.tensor.transpose(
                                    out=tp[:, j, :],
                                    in_=sel[:, :, i0 + j],
                                    identity=idn_t[:])
                            st = sp.tile([128, QB, 128], BF16, tag="st")
                            nc.scalar.copy(
                                st[:, 0:nb_, :].rearrange("p a d -> p (a d)"),
                                tp[:, 0:nb_, :].rearrange("p a d -> p (a d)"))
                            qps = qpp.tile([128, QB, D], F32, tag="qps")
                            for j in range(nb_):
                                for (blk, lo, hi, _f, _l, _c) in \
                                        sl["pieces"][i0 + j]:
                                    nc.tensor.matmul(
                                        out=qps[lo:hi, j, :],
                                        lhsT=st[:, j, lo:hi],
                                        rhs=q_all[:, blk, :],
                                        start=True, stop=True)
                            nc.vector.tensor_tensor(
                                out=qk[:, i0:i0 + nb_, :],
                                in0=qps[:, 0:nb_, :],
                                in1=kvg[:, i0:i0 + nb_, 0:D],
                                op=mybir.AluOpType.mult)
                        state[t0] = (mt, kvg, sel, qk)

                    def slab_C(sl):
                        c, t0, T = sl["chunk"], sl["t0"], sl["T"]
                        mt, kvg, sel, qk = state.pop(t0)
                        ew_v = mt[:, 9 * T:13 * T].bitcast(BF16) \
                            .rearrange("p (t h) -> p t h", h=H)

                        t1 = wp.tile([128, SLAB_T, H, HD // 2], BF16, tag="t1")
                        qk4 = qk.rearrange("p t (h d) -> p t h d", h=H)
                        nc.vector.tensor_tensor(
                            out=t1[:, 0:T, :, :],
                            in0=qk4[:, 0:T, :, 0:HD // 2],
                            in1=qk4[:, 0:T, :, HD // 2:HD],
                            op=mybir.AluOpType.add)
                        t2 = wp.tile([128, SLAB_T, H, HD // 4], BF16, tag="t2")
                        nc.vector.tensor_tensor(
                            out=t2[:, 0:T, :, :],
                            in0=t1[:, 0:T, :, 0:HD // 4],
                            in1=t1[:, 0:T, :, HD // 4:HD // 2],
                            op=mybir.AluOpType.add)
                        sc = wp.tile([128, SLAB_T, H], F32, tag="sc")
                        nc.vector.tensor_reduce(
                            out=sc[:, 0:T, :], in_=t2[:, 0:T, :, :],
                            axis=mybir.AxisListType.X, op=mybir.AluOpType.add)
                        ws = wp.tile([128, SLAB_T, H], F32, tag="ws")
                        nc.gpsimd.tensor_tensor(
                            out=ws[:, 0:T, :], in0=sc[:, 0:T, :],
                            in1=ew_v, op=mybir.AluOpType.mult)
                        ex = wp.tile([128, SLAB_T, H], BF16, tag="ex")
                        nc.scalar.activation(
                            out=ex[:, 0:T, :], in_=ws[:, 0:T, :],
                            func=mybir.ActivationFunctionType.Exp)

                        ctb = wp.tile([128, SLAB_T, D + H], BF16, tag="ctb")
                        nc.vector.tensor_copy(
                            ctb[:, 0:T, D:D + H], ex[:, 0:T, :])
                        nc.vector.tensor_tensor(
                            out=ctb[:, 0:T, 0:D]
                                .rearrange("p t (e h) -> p t e h", h=H),
                            in0=kvg[:, 0:T, D:2 * D]
                                .rearrange("p t (e h) -> p t e h", h=H),
                            in1=ex[:, 0:T, :]
                                .rearrange("p t (o h) -> p t o h", o=1)
                                .to_broadcast([128, T, HD, H]),
                            op=mybir.AluOpType.mult)

                        # segment-sum into per-cell psum, flush to SBUF accum
                        for i in range(T):
                            for (blk, lo, hi, first, last, cell) in \
                                    sl["pieces"][i]:
                                if first:
                                    live[cell] = bpp.tile(
                                        [128, D + H], F32, tag="bps",
                                        name=f"bps{cell}")
                                bps = live[cell]
                                nc.tensor.matmul(
                                    out=bps[:],
                                    lhsT=sel[lo:hi, :, i],
                                    rhs=ctb[lo:hi, i, :],
                                    start=first, stop=last)
                                if last:
                                    nc.vector.tensor_tensor(
                                        out=acc[:, blk, :],
                                        in0=acc[:, blk, :],
                                        in1=bps[:], op=mybir.AluOpType.add)
                                    del live[cell]

                    # ---- pipeline: build chunk 0, then A/B/C software
                    # pipeline over slabs with chunk c+1 builds interleaved
                    NGRP = (CHROWS + 2047) // 2048
                    for g in range(NGRP):
                        build_kv_group(0, g)
                    ns = len(slabs)
                    # build-group schedule: spread chunk c+1's NGRP groups
                    # over chunk c's A-stage iterations
                    binj = {}
                    for c in range(CH - 1):
                        idxs = [i for i, s in enumerate(slabs)
                                if s["chunk"] == c]
                        for j, g in enumerate(range(NGRP)):
                            at = idxs[min(j * len(idxs) // NGRP,
                                          len(idxs) - 1)]
                            binj.setdefault(at, []).append((c + 1, g))
                    for k in range(ns + 2):
                        if k - 2 >= 0:
                            slab_C(slabs[k - 2])
                        if k < ns:
                            for (bc, bg) in binj.get(k, []):
                                build_kv_group(bc, bg)
                            slab_A(slabs[k])
                        if 0 <= k - 1 < ns:
                            slab_B(slabs[k - 1])

                # ---- epilogue over all blocks
                with (
                    tc.tile_pool(name="ep", bufs=2) as ep,
                    tc.tile_pool(name="ett", bufs=2, space="PSUM") as et,
                    tc.tile_pool(name="eo", bufs=2, space="PSUM") as eo,
                ):
                    EB = 14                       # blocks per epilogue slab
                    for s0 in range(0, NBLK, EB):
                        nb = min(EB, NBLK - s0)
                        zr = ep.tile([128, EB, H], F32, tag="zr")
                        nc.vector.tensor_scalar_add(
                            zr[:, 0:nb, :], acc[:, s0:s0 + nb, D:D + H],
                            1e-16)
                        nc.vector.reciprocal(zr[:, 0:nb, :], zr[:, 0:nb, :])
                        vals = ep.tile([128, EB, D], F32, tag="vals")
                        nc.vector.tensor_tensor(
                            out=vals[:, 0:nb, :]
                                .rearrange("p b (e h) -> p b e h", h=H),
                            in0=acc[:, s0:s0 + nb, 0:D]
                                .rearrange("p b (e h) -> p b e h", h=H),
                            in1=zr[:, 0:nb, :]
                                .rearrange("p b (o h) -> p b o h", o=1)
                                .to_broadcast([128, nb, HD, H]),
                            op=mybir.AluOpType.mult)
                        po = eo.tile([128, EB, D], F32, tag="po")
                        for i in range(nb):
                            pt = et.tile([D, 128], F32, tag="pt")
                            nc.tensor.transpose(
                                out=pt[:], in_=vals[:, i, :],
                                identity=idnf_t[:])
                            vT = ep.tile([D, 128], BF16, tag="vT")
                            nc.vector.tensor_copy(vT[:], pt[:])
                            nc.tensor.matmul(
                                out=po[:, i, :], lhsT=vT[:], rhs=wot_t[:],
                                start=True, stop=True)
                        nmu = ep.tile([128, EB], F32, tag="nmu")
                        nc.vector.tensor_reduce(
                            out=nmu[:, 0:nb], in_=po[:, 0:nb, :],
                            axis=mybir.AxisListType.X, op=mybir.AluOpType.add)
                        nc.vector.tensor_scalar_mul(
                            nmu[:, 0:nb], nmu[:, 0:nb], -1.0 / D)
                        ct = ep.tile([128, EB, D], F32, tag="ct")
                        nc.vector.tensor_tensor(
                            out=ct[:, 0:nb, :], in0=po[:, 0:nb, :],
                            in1=nmu[:, 0:nb].rearrange("p (b o) -> p b o", o=1)
                                .to_broadcast([128, nb, D]),
                            op=mybir.AluOpType.add)
                        nc.gpsimd.tensor_tensor(
                            out=ct[:, 0:nb, :], in0=ct[:, 0:nb, :],
                            in1=boc_t[:].rearrange("p (o d) -> p o d", o=1)
                                .to_broadcast([128, nb, D]),
                            op=mybir.AluOpType.add)
                        sq = ep.tile([128, EB, D], F32, tag="sq")
                        nc.gpsimd.tensor_tensor(
                            out=sq[:, 0:nb, :], in0=ct[:, 0:nb, :],
                            in1=ct[:, 0:nb, :], op=mybir.AluOpType.mult)
                        v1 = ep.tile([128, EB], F32, tag="v1")
                        nc.vector.tensor_reduce(
                            out=v1[:, 0:nb], in_=sq[:, 0:nb, :],
                            axis=mybir.AxisListType.X, op=mybir.AluOpType.add)
                        nc.vector.tensor_scalar(
                            out=v1[:, 0:nb], in0=v1[:, 0:nb],
                            scalar1=1.0 / D, scalar2=LN_EPS,
                            op0=mybir.AluOpType.mult,
                            op1=mybir.AluOpType.add)
                        nc.vector.reciprocal(v1[:, 0:nb], v1[:, 0:nb])
                        rstd = ep.tile([128, EB], F32, tag="rstd")
                        nc.scalar.sqrt(rstd[:, 0:nb], v1[:, 0:nb])
                        xb = ep.tile([128, EB, D], BF16, tag="xb")
                        nc.sync.dma_start(
                            out=xb[:, 0:nb, :],
                            in_=xpb[s0 * 128:(s0 + nb) * 128, :]
                                .rearrange("(a p) d -> p a d", p=128))
                        ot = ep.tile([128, EB, D], F32, tag="ot")
                        nc.vector.tensor_tensor(
                            out=ot[:, 0:nb, :], in0=ct[:, 0:nb, :],
                            in1=rstd[:, 0:nb]
                                .rearrange("p (b o) -> p b o", o=1)
                                .to_broadcast([128, nb, D]),
                            op=mybir.AluOpType.mult)
                        nc.gpsimd.tensor_tensor(
                            out=ot[:, 0:nb, :], in0=ot[:, 0:nb, :],
                            in1=gam_t[:].rearrange("p (o d) -> p o d", o=1)
                                .to_broadcast([128, nb, D]),
                            op=mybir.AluOpType.mult)
                        nc.gpsimd.tensor_tensor(
                            out=ot[:, 0:nb, :], in0=ot[:, 0:nb, :],
                            in1=xb[:, 0:nb, :], op=mybir.AluOpType.add)
                        nc.sync.dma_start(
                            out=out[s0 * 128:(s0 + nb) * 128, :]
                                .rearrange("(a p) d -> p a d", p=128),
                            in_=ot[:, 0:nb, :])
    return nc


def kernel(x, edge_index, edge_weights, Wq, bq, Wk, bk, Wv, bv, Wo, bo,
           gamma, beta):
    x = np.asarray(x, np.float32)
    edge_index = np.asarray(edge_index)
    edge_weights = np.asarray(edge_weights, np.float32)
    origins = np.asarray(edge_index[0], np.int64)
    dests = np.asarray(edge_index[1], np.int64)

    struct, per_core_raw = _build_structure(origins, dests)
    nc = _build_graph(struct)
    nc.finalize()

    xT = np.zeros((D + 1, NT), np.float32)
    xT[:D, :N] = x.T
    xT[D] = 1.0
    xT = xT.astype(BF16_NP)
    vperm = (np.arange(H)[None, :] * HD + np.arange(HD)[:, None]).ravel()
    wkv = np.zeros((D + 1, 2 * D), np.float32)
    wkv[:D, :D] = np.asarray(Wk, np.float32).T
    wkv[:D, D:] = np.asarray(Wv, np.float32).T[:, vperm]
    wkv[D, :D] = np.asarray(bk, np.float32)
    wkv[D, D:] = np.asarray(bv, np.float32)[vperm]
    wq = np.zeros((D + 1, D), np.float32)
    wq[:D, :] = np.asarray(Wq, np.float32).T
    wq[D, :] = np.asarray(bq, np.float32)
    wkv = wkv.astype(BF16_NP)
    wq = wq.astype(BF16_NP)
    wot = np.ascontiguousarray(np.asarray(Wo, np.float32).T[vperm, :])
    bo_ = np.asarray(bo, np.float32)
    boc = np.tile((bo_ - bo_.mean())[None, :], (128, 1)).astype(np.float32)
    gam_t = np.tile(np.asarray(gamma, np.float32)[None, :], (128, 1))
    iot = np.tile(np.arange(128, dtype=np.float32)[None, :, None],
                  (128, 1, SLAB_T)).astype(BF16_NP)
    idn = np.eye(128, dtype=np.float32)

    in_maps = []
    for c in range(NCORES):
        data = _per_core_arrays(struct, per_core_raw[c], edge_weights)
        xTo = np.zeros((D + 1, NB), np.float32)
        xTo[:D, :NOWN] = x[c * NOWN:(c + 1) * NOWN].T
        xTo[D] = 1.0
        xTo = xTo.astype(BF16_NP)
        xpb = np.zeros((NB, D), np.float32)
        xpb[:NOWN] = x[c * NOWN:(c + 1) * NOWN] + np.asarray(beta, np.float32)
        in_maps.append({
            "xT": xT, "xTo": xTo, "wkv": wkv, "wq": wq, "wot": wot,
            "boc": boc, "gam": gam_t, "iot": iot,
            "idn": idn.astype(BF16_NP), "idnf": idn,
            "xpb": xpb.astype(BF16_NP), "met": data["meta"],
        })

    global LAST_SIM_NS
    if SIMULATE_COST:
        from concourse import bass_interp
        sim = bass_interp.CoreSim(nc, no_exec=True, publish_trace=False)
        sim.event_loop()
        LAST_SIM_NS = int(sim.time)

    res = run_bass_kernel_spmd(nc, in_maps, core_ids=list(range(NCORES)),
                               trace=TRACE)
    global LAST_RESULT
    LAST_RESULT = res
    outs = [np.asarray(res.results[i]["out"])[:NOWN] for i in range(NCORES)]
    return np.concatenate(outs, axis=0).astype(np.float32)


TRACE = False
SIMULATE_COST = False
LAST_RESULT = None
LAST_SIM_NS = None


# revision 3
# speedup vs baseline: 1.3004x; 1.3004x over previous
"""Trainium2 Bass kernel v2 for BaseDependentAttentionLayer (GNN message passing).

Design vs v1 baseline (739us cost-model):
  - Chunk-outer pipeline: kv-table chunk c+1 builds (PE/DMA) while chunk c's
    edges process (DVE/Pool/Act).  Removes the 175us serial phase-1 stall.
  - No q gather: per tile, PE transposes the sel one-hot (slot x origin),
    Act copies selT to SBUF, PE matmul broadcasts the 128-node q block to
    slots.  Kills 250k gather descs of DMA + Pool desc-gen.
  - Per-block accumulators live in SBUF f32 across all 4 chunks; single
    final epilogue (one act-table switch, big-slab instructions).
  - Metadata packed per slab in one i16 stream (kvi | oid | ew-bf16).
  - Engine placement: sel/qk/tree/reduce/ctb on DVE; ws + psum flushes +
    phase1 copies on Pool; selT copies + exp on Act; DMA issues on SP.
"""

import sys

sys.path.insert(0, "/opt/trn_rl_repo")

import numpy as np
import ml_dtypes

import concourse.bass as bass
import concourse.bacc as bacc
import concourse.mybir as mybir
from concourse.tile import TileContext
from concourse.bass_utils import run_bass_kernel_spmd

N = 100000
E = 1600000
D = 64
H = 4
HD = 16
NCORES = 8
NOWN = 12500            # nodes owned per core
NBLK = 98               # 128-node blocks per core
NB = NBLK * 128         # 12544 padded own nodes
NT = 100352             # padded global table rows (= 4 * 25088)
CH = 4                  # dest chunks (int16 gather index limit)
CHROWS = NT // CH       # 25088
SLAB_T = 48             # edge tiles per processing slab
QB = 8                  # tiles per selT/q-broadcast batch
LN_EPS = 1e-5
PAD_OID = 200.0

F32 = mybir.dt.float32
BF16 = mybir.dt.bfloat16
I16 = mybir.dt.int16
BF16_NP = ml_dtypes.bfloat16


def _build_structure(origins, dests):
    """Chunk-major packed cell layout (no per-cell 128-rounding).

    Cells (chunk, block) are packed back-to-back; only chunk streams are
    padded to 128.  Tiles spanning cell boundaries are handled with
    partition-sliced matmuls ("pieces").  Structure is shared by cores:
    per-cell slot count = max over cores.
    """
    owner = origins // NOWN
    per_core_raw = []
    cnts = np.zeros((NCORES, CH * NBLK), np.int64)
    for c in range(NCORES):
        m = owner == c
        o = (origins[m] - c * NOWN).astype(np.int32)
        d = dests[m].astype(np.int32)
        eids = np.nonzero(m)[0]
        blk = o >> 7
        chunk = d // CHROWS
        cell = chunk * NBLK + blk          # chunk-major
        order = np.argsort(cell, kind="stable")
        o, d, eids = o[order], d[order], eids[order]
        cnt = np.bincount(cell[order], minlength=CH * NBLK)
        cnts[c] = cnt
        per_core_raw.append((o, d, eids, cnt))
    # 128-aligned cells: base-64 matmul pieces compile but fail at runtime
    cell_slots = ((cnts.max(0) + 127) // 128) * 128

    cell_off = np.zeros(CH * NBLK, np.int64)
    pieces_by_tile = {}
    tile_total = 0
    chunk_ranges = []                      # (tile0, ntiles) per chunk
    for c in range(CH):
        s0 = tile_total * 128
        pos = s0
        for b in range(NBLK):
            cell = c * NBLK + b
            cell_off[cell] = pos
            n = int(cell_slots[cell])
            if n == 0:
                continue
            p0 = pos
            while p0 < pos + n:
                t = p0 // 128
                lo = p0 - t * 128
                hi = min(128, pos + n - t * 128)
                pieces_by_tile.setdefault(t, []).append(
                    (b, lo, hi, p0 == pos, t * 128 + hi == pos + n, cell))
                p0 = t * 128 + hi
            pos += n
        ntiles = (pos - s0 + 127) // 128
        chunk_ranges.append((tile_total, ntiles))
        tile_total += ntiles
    S_tiles = tile_total

    slabs = []
    meta_off = 0
    for c in range(CH):
        t0c, tc = chunk_ranges[c]
        s = t0c
        while s < t0c + tc:
            T = min(SLAB_T, t0c + tc - s)
            slabs.append({
                "chunk": c, "t0": s, "T": T, "meta_off": meta_off,
                "pieces": {t - s: pieces_by_tile.get(t, [])
                           for t in range(s, s + T)},
            })
            meta_off += 13 * T
            s += T
    struct = {
        "slabs": slabs,
        "S_tiles": S_tiles,
        "meta_cols": meta_off,
        "cell_slots": cell_slots,
        "cell_off": cell_off,
    }
    return struct, per_core_raw


def _per_core_arrays(struct, core_raw, edge_weights):
    """Packed per-slab metadata stream [128, meta_cols] int16."""
    o, d, eids, cnt = core_raw
    S_tiles = struct["S_tiles"]
    S = S_tiles * 128
    cell_off = struct["cell_off"]

    oid = np.full(S, PAD_OID, np.float32)
    kvi = np.zeros(S, np.int16)
    ew4 = np.zeros((S, H), np.float32)

    cell_edge_off = np.zeros(CH * NBLK + 1, np.int64)
    np.cumsum(cnt, out=cell_edge_off[1:])
    for cell in range(CH * NBLK):
        n = int(cnt[cell])
        if n == 0:
            continue
        e0 = cell_edge_off[cell]
        s0 = int(cell_off[cell])
        ch = cell // NBLK
        sl = slice(s0, s0 + n)
        el = slice(e0, e0 + n)
        oid[sl] = (o[el] & 127).astype(np.float32)
        kvi[sl] = (d[el] - ch * CHROWS).astype(np.int16)
        ew4[sl] = edge_weights[eids[el]] * (HD ** -0.5)

    # slot-major -> tile layouts
    oid_t = np.ascontiguousarray(oid.reshape(S_tiles, 128).T).astype(BF16_NP)
    ew_t = np.ascontiguousarray(
        ew4.reshape(S_tiles, 128, H).transpose(1, 0, 2)).astype(BF16_NP)

    def wrap(run_vals):
        w = run_vals.reshape(-1, 16).T          # [16, len/16]
        return np.tile(w, (8, 1))               # [128, len/16]

    meta = np.zeros((128, struct["meta_cols"]), np.int16)
    for sl in struct["slabs"]:
        t0, T, mo = sl["t0"], sl["T"], sl["meta_off"]
        meta[:, mo:mo + 8 * T] = wrap(kvi[t0 * 128:(t0 + T) * 128])
        meta[:, mo + 8 * T:mo + 9 * T] = oid_t[:, t0:t0 + T].view(np.int16)
        meta[:, mo + 9 * T:mo + 13 * T] = (
            ew_t[:, t0:t0 + T, :].reshape(128, T * H).view(np.int16))
    return {"meta": meta}


def _build_graph(struct):
    nc = bacc.Bacc()
    slabs = struct["slabs"]

    xT = nc.declare_dram_parameter("xT", [D + 1, NT], BF16, isOutput=False)
    xTo = nc.declare_dram_parameter("xTo", [D + 1, NB], BF16, isOutput=False)
    wkv = nc.declare_dram_parameter("wkv", [D + 1, 2 * D], BF16, isOutput=False)
    wq = nc.declare_dram_parameter("wq", [D + 1, D], BF16, isOutput=False)
    wot = nc.declare_dram_parameter("wot", [D, D], F32, isOutput=False)
    boc = nc.declare_dram_parameter("boc", [128, D], F32, isOutput=False)
    gam = nc.declare_dram_parameter("gam", [128, D], F32, isOutput=False)
    iot = nc.declare_dram_parameter("iot", [128, 128, SLAB_T], BF16,
                                    isOutput=False)
    idn = nc.declare_dram_parameter("idn", [128, 128], BF16, isOutput=False)
    idnf = nc.declare_dram_parameter("idnf", [128, 128], F32, isOutput=False)
    xpb = nc.declare_dram_parameter("xpb", [NB, D], BF16, isOutput=False)
    met = nc.declare_dram_parameter("met", [128, struct["meta_cols"]], I16,
                                    isOutput=False)
    out = nc.declare_dram_parameter("out", [NB, D], F32, isOutput=True)

    kv_tab = nc.dram_tensor("kv_tab", [NT, 2 * D], BF16)

    with TileContext(nc) as tc:
        with tc.tile_pool(name="const", bufs=1) as cp:
            wkv_t = cp.tile([D + 1, 2 * D], BF16)
            nc.sync.dma_start(out=wkv_t[:], in_=wkv[:])
            wq_t = cp.tile([D + 1, D], BF16)
            nc.sync.dma_start(out=wq_t[:], in_=wq[:])
            wot_f = cp.tile([D, D], F32)
            nc.sync.dma_start(out=wot_f[:], in_=wot[:])
            wot_t = cp.tile([D, D], BF16)
            nc.vector.tensor_copy(wot_t[:], wot_f[:])
            boc_t = cp.tile([128, D], F32)
            nc.sync.dma_start(out=boc_t[:], in_=boc[:])
            gam_t = cp.tile([128, D], F32)
            nc.sync.dma_start(out=gam_t[:], in_=gam[:])
            iot_t = cp.tile([128, 128, SLAB_T], BF16)
            nc.sync.dma_start(out=iot_t[:], in_=iot[:])
            idn_t = cp.tile([128, 128], BF16)
            nc.sync.dma_start(out=idn_t[:], in_=idn[:])
            idnf_t = cp.tile([128, 128], F32)
            nc.sync.dma_start(out=idnf_t[:], in_=idnf[:])

            # q blocks for all own nodes, SBUF-resident [128, NBLK, D] bf16
            q_all = cp.tile([128, NBLK, D], BF16)
            # per-block f32 accumulators [128, NBLK, D+H]
            acc = cp.tile([128, NBLK, D + H], F32)
            nc.gpsimd.memset(acc[:], 0.0)

            # ---- q blocks: lhsT = xTo block cols, rhs = wq -> [128, 64]
            with (
                tc.tile_pool(name="qbx", bufs=2) as qbx,
                tc.tile_pool(name="qbp", bufs=2, space="PSUM") as qbp,
            ):
                for bat in range((NBLK + 7) // 8):
                    b0 = bat * 8
                    nb = min(8, NBLK - b0)
                    xs = qbx.tile([D + 1, 8 * 128], BF16, tag="xq")
                    nc.sync.dma_start(
                        out=xs[:, 0:nb * 128],
                        in_=xTo[:, b0 * 128:(b0 + nb) * 128])
                    ps = qbp.tile([128, 8, D], F32, tag="qp")
                    for j in range(nb):
                        nc.tensor.matmul(
                            out=ps[:, j, :],
                            lhsT=xs[:, j * 128:(j + 1) * 128],
                            rhs=wq_t[:],
                            start=True, stop=True)
                    nc.vector.tensor_copy(
                        q_all[:, b0:b0 + nb, :], ps[:, 0:nb, :])

            with (
                tc.tile_pool(name="p1x", bufs=2) as p1x,
                tc.tile_pool(name="p1s", bufs=2) as p1s,
                tc.tile_pool(name="p1p", bufs=2, space="PSUM") as p1p,
            ):
                # ---- kv table chunk builder: groups of 16 blocks (2048 rows)
                def build_kv_group(c, g):
                    # chunk 0 builds serially at startup while Act/SP are
                    # otherwise idle: split load/write across both queues
                    ld = nc.sync
                    wr = nc.sync
                    r0 = c * CHROWS + g * 2048
                    nrow = min(2048, CHROWS - g * 2048)
                    nblk4 = (nrow + 511) // 512     # 4-block psum batches
                    xs = p1x.tile([D + 1, 2048], BF16, tag="xs")
                    ld.dma_start(
                        out=xs[:, 0:nrow], in_=xT[:, r0:r0 + nrow])
                    sb = p1s.tile([128, 16, 2 * D], BF16, tag="sb")
                    for bat in range(nblk4):
                        ps = p1p.tile([128, 4, 2 * D], F32, tag="kp")
                        for j in range(4):
                            nc.tensor.matmul(
                                out=ps[:, j, :],
                                lhsT=xs[:, bat * 512 + j * 128:
                                        bat * 512 + (j + 1) * 128],
                                rhs=wkv_t[:],
                                start=True, stop=True)
                        if c == 0 and bat % 2 == 0:
                            nc.vector.tensor_copy(
                                sb[:, bat * 4:(bat + 1) * 4, :]
                                    .rearrange("p a d -> p (a d)"),
                                ps[:].rearrange("p a d -> p (a d)"))
                        else:
                            nc.scalar.copy(
                                sb[:, bat * 4:(bat + 1) * 4, :]
                                    .rearrange("p a d -> p (a d)"),
                                ps[:].rearrange("p a d -> p (a d)"))
                    wr.dma_start(
                        out=kv_tab[r0:r0 + nrow, :]
                            .rearrange("(a p) d -> p a d", p=128),
                        in_=sb[:, 0:nrow // 128, :])

                # ---- edge-processing slab
                with (
                    tc.tile_pool(name="gat", bufs=3) as gp,
                    tc.tile_pool(name="mp", bufs=3) as mp,
                    tc.tile_pool(name="sp", bufs=2) as sp,
                    tc.tile_pool(name="qs", bufs=3) as qp,
                    tc.tile_pool(name="wk", bufs=2) as wp,
                    tc.tile_pool(name="stp", bufs=2, space="PSUM") as stp,
                    tc.tile_pool(name="qpp", bufs=2, space="PSUM") as qpp,
                    tc.tile_pool(name="bpp", bufs=2, space="PSUM") as bpp,
                ):
                    state = {}
                    live = {}

                    def slab_A(sl):
                        c, t0, T = sl["chunk"], sl["t0"], sl["T"]
                        mo = sl["meta_off"]
                        mt = mp.tile([128, 13 * SLAB_T], I16, tag="mt")
                        nc.sync.dma_start(
                            out=mt[:, 0:13 * T], in_=met[:, mo:mo + 13 * T])
                        kvi_v = mt[:, 0:8 * T]
                        kvg = gp.tile([128, SLAB_T, 2 * D], BF16, tag="kvg")
                        nc.gpsimd.dma_gather(
                            out_ap=kvg[:, 0:T, :],
                            in_ap=kv_tab[c * CHROWS:(c + 1) * CHROWS, :],
                            idxs_ap=kvi_v,
                            num_idxs=T * 128,
                            num_idxs_reg=T * 128,
                            elem_size=2 * D,
                            single_packet=False)
                        state[sl["t0"]] = (mt, kvg)

                    def slab_B(sl):
                        c, t0, T = sl["chunk"], sl["t0"], sl["T"]
                        mt, kvg = state[t0]
                        oid_v = mt[:, 8 * T:9 * T].bitcast(BF16)

                        sel = sp.tile([128, 128, SLAB_T], BF16, tag="sel")
                        nc.vector.tensor_tensor(
                            out=sel[:, :, 0:T],
                            in0=oid_v.rearrange("p (o t) -> p o t", o=1)
                                .to_broadcast([128, 128, T]),
                            in1=iot_t[:, :, 0:T],
                            op=mybir.AluOpType.is_equal)

                        # q broadcast: transpose sel -> selT, q = selT^T @ Q,
                        # qk = q*k straight from PSUM (gpsimd can't read PSUM)
                        qk = qp.tile([128, SLAB_T, D], BF16, tag="qk")
                        nbat = (T + QB - 1) // QB
                        for b in range(nbat):
                            i0 = b * QB
                            nb_ = min(QB, T - i0)
                            tp = stp.tile([128, QB, 128], BF16, tag="tp")
                            for j in range(nb_):
                                nc.tensor.transpose(
                                    out=tp[:, j, :],
                                    in_=sel[:, :, i0 + j],
                                    identity=idn_t[:])
                            st = sp.tile([128, QB, 128], BF16, tag="st")
                            nc.scalar.copy(
                                st[:, 0:nb_, :].rearrange("p a d -> p (a d)"),
                                tp[:, 0:nb_, :].rearrange("p a d -> p (a d)"))
                            qps = qpp.tile([128, QB, D], F32, tag="qps")
                            for j in range(nb_):
                                for (blk, lo, hi, _f, _l, _c) in \
                                        sl["pieces"][i0 + j]:
                                    nc.tensor.matmul(
                                        out=qps[lo:hi, j, :],
                                        lhsT=st[:, j, lo:hi],
                                        rhs=q_all[:, blk, :],
                                        start=True, stop=True)
                            nc.vector.tensor_tensor(
                                out=qk[:, i0:i0 + nb_, :],
                                in0=qps[:, 0:nb_, :],
                                in1=kvg[:, i0:i0 + nb_, 0:D],
                                op=mybir.AluOpType.mult)
                        state[t0] = (mt, kvg, sel, qk)

                    def slab_C(sl):
                        c, t0, T = sl["chunk"], sl["t0"], sl["T"]
                        mt, kvg, sel, qk = state.pop(t0)
                        ew_v = mt[:, 9 * T:13 * T].bitcast(BF16) \
                            .rearrange("p (t h) -> p t h", h=H)

                        t1 = wp.tile([128, SLAB_T, H, HD // 2], BF16, tag="t1")
                        qk4 = qk.rearrange("p t (h d) -> p t h d", h=H)
                        nc.vector.tensor_tensor(
                            out=t1[:, 0:T, :, :],
                            in0=qk4[:, 0:T, :, 0:HD // 2],
                            in1=qk4[:, 0:T, :, HD // 2:HD],
                            op=mybir.AluOpType.add)
                        t2 = wp.tile([128, SLAB_T, H, HD // 4], BF16, tag="t2")
                        nc.vector.tensor_tensor(
                            out=t2[:, 0:T, :, :],
                            in0=t1[:, 0:T, :, 0:HD // 4],
                            in1=t1[:, 0:T, :, HD // 4:HD // 2],
                            op=mybir.AluOpType.add)
                        t3 = wp.tile([128, SLAB_T, H, HD // 8], BF16, tag="t3")
                        nc.vector.tensor_tensor(
                            out=t3[:, 0:T, :, :],
                            in0=t2[:, 0:T, :, 0:HD // 8],
                            in1=t2[:, 0:T, :, HD // 8:HD // 4],
                            op=mybir.AluOpType.add)
                        sc = wp.tile([128, SLAB_T, H], F32, tag="sc")
                        nc.vector.tensor_reduce(
                            out=sc[:, 0:T, :], in_=t3[:, 0:T, :, :],
                            axis=mybir.AxisListType.X, op=mybir.AluOpType.add)
                        ws = wp.tile([128, SLAB_T, H], F32, tag="ws")
                        nc.gpsimd.tensor_tensor(
                            out=ws[:, 0:T, :], in0=sc[:, 0:T, :],
                            in1=ew_v, op=mybir.AluOpType.mult)
                        ex = wp.tile([128, SLAB_T, H], BF16, tag="ex")
                        nc.scalar.activation(
                            out=ex[:, 0:T, :], in_=ws[:, 0:T, :],
                            func=mybir.ActivationFunctionType.Exp)

                        ctb = wp.tile([128, SLAB_T, D + H], BF16, tag="ctb")
                        nc.vector.tensor_copy(
                            ctb[:, 0:T, D:D + H], ex[:, 0:T, :])
                        nc.gpsimd.tensor_tensor(
                            out=ctb[:, 0:T, 0:D]
                                .rearrange("p t (e h) -> p t e h", h=H),
                            in0=kvg[:, 0:T, D:2 * D]
                                .rearrange("p t (e h) -> p t e h", h=H),
                            in1=ex[:, 0:T, :]
                                .rearrange("p t (o h) -> p t o h", o=1)
                                .to_broadcast([128, T, HD, H]),
                            op=mybir.AluOpType.mult)

                        # segment-sum into per-cell psum slices; flush 4
                        # consecutive blocks per DVE add (amortize PSUM init)
                        for i in range(T):
                            for (blk, lo, hi, first, last, cell) in \
                                    sl["pieces"][i]:
                                grp = blk // 4
                                key = (c, grp)
                                if key not in live:
                                    live[key] = bpp.tile(
                                        [128, 4, D + H], F32, tag="bps",
                                        name=f"bps{c}_{grp}")
                                bps = live[key]
                                nc.tensor.matmul(
                                    out=bps[:, blk % 4, :],
                                    lhsT=sel[lo:hi, :, i],
                                    rhs=ctb[lo:hi, i, :],
                                    start=first, stop=last)
                                if last and blk == min(4 * grp + 3, NBLK - 1):
                                    nb_g = blk - 4 * grp + 1
                                    nc.vector.tensor_tensor(
                                        out=acc[:, 4 * grp:blk + 1, :],
                                        in0=acc[:, 4 * grp:blk + 1, :],
                                        in1=bps[:, 0:nb_g, :],
                                        op=mybir.AluOpType.add)
                                    del live[key]

                    # ---- pipeline: build chunk 0, then A/B/C software
                    # pipeline over slabs with chunk c+1 builds interleaved
                    NGRP = (CHROWS + 2047) // 2048
                    for g in range(NGRP):
                        build_kv_group(0, g)
                    ns = len(slabs)
                    # build-group schedule: spread chunk c+1's NGRP groups
                    # over chunk c's A-stage iterations
                    binj = {}
                    for c in range(CH - 1):
                        idxs = [i for i, s in enumerate(slabs)
                                if s["chunk"] == c]
                        for j, g in enumerate(range(NGRP)):
                            at = idxs[min(j * len(idxs) // NGRP,
                                          len(idxs) - 1)]
                            binj.setdefault(at, []).append((c + 1, g))
                    for k in range(ns + 2):
                        if k - 2 >= 0:
                            slab_C(slabs[k - 2])
                        if k < ns:
                            for (bc, bg) in binj.get(k, []):
                                build_kv_group(bc, bg)
                            slab_A(slabs[k])
                        if 0 <= k - 1 < ns:
                            slab_B(slabs[k - 1])

                # ---- epilogue over all blocks
                with (
                    tc.tile_pool(name="ep", bufs=2) as ep,
                    tc.tile_pool(name="ett", bufs=2, space="PSUM") as et,
                    tc.tile_pool(name="eo", bufs=2, space="PSUM") as eo,
                ):
                    EB = 14                       # blocks per epilogue slab
                    for s0 in range(0, NBLK, EB):
                        nb = min(EB, NBLK - s0)
                        zr = ep.tile([128, EB, H], F32, tag="zr")
                        nc.vector.tensor_scalar_add(
                            zr[:, 0:nb, :], acc[:, s0:s0 + nb, D:D + H],
                            1e-16)
                        nc.vector.reciprocal(zr[:, 0:nb, :], zr[:, 0:nb, :])
                        vals = ep.tile([128, EB, D], F32, tag="vals")
                        nc.gpsimd.tensor_tensor(
                            out=vals[:, 0:nb, :]
                                .rearrange("p b (e h) -> p b e h", h=H),
                            in0=acc[:, s0:s0 + nb, 0:D]
                                .rearrange("p b (e h) -> p b e h", h=H),
                            in1=zr[:, 0:nb, :]
                                .rearrange("p b (o h) -> p b o h", o=1)
                                .to_broadcast([128, nb, HD, H]),
                            op=mybir.AluOpType.mult)
                        po = eo.tile([128, EB, D], F32, tag="po")
                        for i in range(nb):
                            pt = et.tile([D, 128], F32, tag="pt")
                            nc.tensor.transpose(
                                out=pt[:], in_=vals[:, i, :],
                                identity=idnf_t[:])
                            vT = ep.tile([D, 128], BF16, tag="vT")
                            nc.scalar.copy(vT[:], pt[:])
                            nc.tensor.matmul(
                                out=po[:, i, :], lhsT=vT[:], rhs=wot_t[:],
                                start=True, stop=True)
                        nmu = ep.tile([128, EB], F32, tag="nmu")
                        nc.vector.tensor_reduce(
                            out=nmu[:, 0:nb], in_=po[:, 0:nb, :],
                            axis=mybir.AxisListType.X, op=mybir.AluOpType.add)
                        nc.vector.tensor_scalar_mul(
                            nmu[:, 0:nb], nmu[:, 0:nb], -1.0 / D)
                        ct = ep.tile([128, EB, D], F32, tag="ct")
                        nc.vector.tensor_tensor(
                            out=ct[:, 0:nb, :], in0=po[:, 0:nb, :],
                            in1=nmu[:, 0:nb].rearrange("p (b o) -> p b o", o=1)
                                .to_broadcast([128, nb, D]),
                            op=mybir.AluOpType.add)
                        nc.gpsimd.tensor_tensor(
                            out=ct[:, 0:nb, :], in0=ct[:, 0:nb, :],
                            in1=boc_t[:].rearrange("p (o d) -> p o d", o=1)
                                .to_broadcast([128, nb, D]),
                            op=mybir.AluOpType.add)
                        sq = ep.tile([128, EB, D], F32, tag="sq")
                        nc.gpsimd.tensor_tensor(
                            out=sq[:, 0:nb, :], in0=ct[:, 0:nb, :],
                            in1=ct[:, 0:nb, :], op=mybir.AluOpType.mult)
                        v1 = ep.tile([128, EB], F32, tag="v1")
                        nc.vector.tensor_reduce(
                            out=v1[:, 0:nb], in_=sq[:, 0:nb, :],
                            axis=mybir.AxisListType.X, op=mybir.AluOpType.add)
                        nc.vector.tensor_scalar(
                            out=v1[:, 0:nb], in0=v1[:, 0:nb],
                            scalar1=1.0 / D, scalar2=LN_EPS,
                            op0=mybir.AluOpType.mult,
                            op1=mybir.AluOpType.add)
                        nc.vector.reciprocal(v1[:, 0:nb], v1[:, 0:nb])
                        rstd = ep.tile([128, EB], F32, tag="rstd")
                        nc.scalar.sqrt(rstd[:, 0:nb], v1[:, 0:nb])
                        xb = ep.tile([128, EB, D], BF16, tag="xb")
                        nc.sync.dma_start(
                            out=xb[:, 0:nb, :],
                            in_=xpb[s0 * 128:(s0 + nb) * 128, :]
                                .rearrange("(a p) d -> p a d", p=128))
                        ot = ep.tile([128, EB, D], F32, tag="ot")
                        nc.gpsimd.tensor_tensor(
                            out=ot[:, 0:nb, :], in0=ct[:, 0:nb, :],
                            in1=rstd[:, 0:nb]
                                .rearrange("p (b o) -> p b o", o=1)
                                .to_broadcast([128, nb, D]),
                            op=mybir.AluOpType.mult)
                        nc.gpsimd.tensor_tensor(
                            out=ot[:, 0:nb, :], in0=ot[:, 0:nb, :],
                            in1=gam_t[:].rearrange("p (o d) -> p o d", o=1)
                                .to_broadcast([128, nb, D]),
                            op=mybir.AluOpType.mult)
                        nc.gpsimd.tensor_tensor(
                            out=ot[:, 0:nb, :], in0=ot[:, 0:nb, :],
                            in1=xb[:, 0:nb, :], op=mybir.AluOpType.add)
                        nc.sync.dma_start(
                            out=out[s0 * 128:(s0 + nb) * 128, :]
                                .rearrange("(a p) d -> p a d", p=128),
                            in_=ot[:, 0:nb, :])
    return nc


def kernel(x, edge_index, edge_weights, Wq, bq, Wk, bk, Wv, bv, Wo, bo,
           gamma, beta):
    x = np.asarray(x, np.float32)
    edge_index = np.asarray(edge_index)
    edge_weights = np.asarray(edge_weights, np.float32)
    origins = np.asarray(edge_index[0], np.int64)
    dests = np.asarray(edge_index[1], np.int64)

    struct, per_core_raw = _build_structure(origins, dests)
    nc = _build_graph(struct)
    nc.finalize()

    xT = np.zeros((D + 1, NT), np.float32)
    xT[:D, :N] = x.T
    xT[D] = 1.0
    xT = xT.astype(BF16_NP)
    vperm = (np.arange(H)[None, :] * HD + np.arange(HD)[:, None]).ravel()
    wkv = np.zeros((D + 1, 2 * D), np.float32)
    wkv[:D, :D] = np.asarray(Wk, np.float32).T
    wkv[:D, D:] = np.asarray(Wv, np.float32).T[:, vperm]
    wkv[D, :D] = np.asarray(bk, np.float32)
    wkv[D, D:] = np.asarray(bv, np.float32)[vperm]
    wq = np.zeros((D + 1, D), np.float32)
    wq[:D, :] = np.asarray(Wq, np.float32).T
    wq[D, :] = np.asarray(bq, np.float32)
    wkv = wkv.astype(BF16_NP)
    wq = wq.astype(BF16_NP)
    wot = np.ascontiguousarray(np.asarray(Wo, np.float32).T[vperm, :])
    bo_ = np.asarray(bo, np.float32)
    boc = np.tile((bo_ - bo_.mean())[None, :], (128, 1)).astype(np.float32)
    gam_t = np.tile(np.asarray(gamma, np.float32)[None, :], (128, 1))
    iot = np.tile(np.arange(128, dtype=np.float32)[None, :, None],
                  (128, 1, SLAB_T)).astype(BF16_NP)
    idn = np.eye(128, dtype=np.float32)

    in_maps = []
    for c in range(NCORES):
        data = _per_core_arrays(struct, per_core_raw[c], edge_weights)
        xTo = np.zeros((D + 1, NB), np.float32)
        xTo[:D, :NOWN] = x[c * NOWN:(c + 1) * NOWN].T
        xTo[D] = 1.0
        xTo = xTo.astype(BF16_NP)
        xpb = np.zeros((NB, D), np.float32)
        xpb[:NOWN] = x[c * NOWN:(c + 1) * NOWN] + np.asarray(beta, np.float32)
        in_maps.append({
            "xT": xT, "xTo": xTo, "wkv": wkv, "wq": wq, "wot": wot,
            "boc": boc, "gam": gam_t, "iot": iot,
            "idn": idn.astype(BF16_NP), "idnf": idn,
            "xpb": xpb.astype(BF16_NP), "met": data["meta"],
        })

    global LAST_SIM_NS
    if SIMULATE_COST:
        from concourse import bass_interp
        sim = bass_interp.CoreSim(nc, no_exec=True, publish_trace=False)
        sim.event_loop()
        LAST_SIM_NS = int(sim.time)

    res = run_bass_kernel_spmd(nc, in_maps, core_ids=list(range(NCORES)),
                               trace=TRACE)
    global LAST_RESULT
    LAST_RESULT = res
    outs = [np.asarray(res.results[i]["out"])[:NOWN] for i in range(NCORES)]
    return np.concatenate(outs, axis=0).astype(np.float32)


TRACE = False
SIMULATE_COST = False
LAST_RESULT = None
LAST_SIM_NS = None


# revision 5
# speedup vs baseline: 1.4127x; 1.0863x over previous
"""Trainium2 Bass kernel v2 for BaseDependentAttentionLayer (GNN message passing).

Design vs v1 baseline (739us cost-model):
  - Chunk-outer pipeline: kv-table chunk c+1 builds (PE/DMA) while chunk c's
    edges process (DVE/Pool/Act).  Removes the 175us serial phase-1 stall.
  - No q gather: per tile, PE transposes the sel one-hot (slot x origin),
    Act copies selT to SBUF, PE matmul broadcasts the 128-node q block to
    slots.  Kills 250k gather descs of DMA + Pool desc-gen.
  - Per-block accumulators live in SBUF f32 across all 4 chunks; single
    final epilogue (one act-table switch, big-slab instructions).
  - Metadata packed per slab in one i16 stream (kvi | oid | ew-bf16).
  - Engine placement: sel/qk/tree/reduce/ctb on DVE; ws + psum flushes +
    phase1 copies on Pool; selT copies + exp on Act; DMA issues on SP.
"""

import sys

sys.path.insert(0, "/opt/trn_rl_repo")

import numpy as np
import ml_dtypes

import concourse.bass as bass
import concourse.bacc as bacc
import concourse.mybir as mybir
from concourse.tile import TileContext
from concourse.bass_utils import run_bass_kernel_spmd

N = 100000
E = 1600000
D = 64
H = 4
HD = 16
NCORES = 8
NOWN = 12500            # nodes owned per core
NBLK = 98               # 128-node blocks per core
NB = NBLK * 128         # 12544 padded own nodes
NT = 100352             # padded global table rows (= 4 * 25088)
CH = 4                  # dest chunks (int16 gather index limit)
# uneven chunks: tiny chunk 0 builds fast so edge processing starts early;
# the big chunks build in the shadow of processing (all rows <= 32767)
CHUNK_BASE = [0, 16384, 45056, 72704]
CHUNK_ROWS = [16384, 28672, 27648, 27648]
SLAB_T = 48             # edge tiles per processing slab
QB = 8                  # tiles per selT/q-broadcast batch
LN_EPS = 1e-5
PAD_OID = 200.0

F32 = mybir.dt.float32
BF16 = mybir.dt.bfloat16
I16 = mybir.dt.int16
BF16_NP = ml_dtypes.bfloat16


def _build_structure(origins, dests):
    """Chunk-major packed cell layout (no per-cell 128-rounding).

    Cells (chunk, block) are packed back-to-back; only chunk streams are
    padded to 128.  Tiles spanning cell boundaries are handled with
    partition-sliced matmuls ("pieces").  Structure is shared by cores:
    per-cell slot count = max over cores.
    """
    owner = origins // NOWN
    per_core_raw = []
    cnts = np.zeros((NCORES, CH * NBLK), np.int64)
    for c in range(NCORES):
        m = owner == c
        o = (origins[m] - c * NOWN).astype(np.int32)
        d = dests[m].astype(np.int32)
        eids = np.nonzero(m)[0]
        blk = o >> 7
        chunk = np.searchsorted(np.asarray(CHUNK_BASE), d, side="right") - 1
        cell = chunk * NBLK + blk          # chunk-major
        order = np.argsort(cell, kind="stable")
        o, d, eids = o[order], d[order], eids[order]
        cnt = np.bincount(cell[order], minlength=CH * NBLK)
        cnts[c] = cnt
        per_core_raw.append((o, d, eids, cnt))
    # 128-aligned cells: base-64 matmul pieces compile but fail at runtime
    cell_slots = ((cnts.max(0) + 127) // 128) * 128

    cell_off = np.zeros(CH * NBLK, np.int64)
    pieces_by_tile = {}
    tile_total = 0
    chunk_ranges = []                      # (tile0, ntiles) per chunk
    for c in range(CH):
        s0 = tile_total * 128
        pos = s0
        for b in range(NBLK):
            cell = c * NBLK + b
            cell_off[cell] = pos
            n = int(cell_slots[cell])
            if n == 0:
                continue
            p0 = pos
            while p0 < pos + n:
                t = p0 // 128
                lo = p0 - t * 128
                hi = min(128, pos + n - t * 128)
                pieces_by_tile.setdefault(t, []).append(
                    (b, lo, hi, p0 == pos, t * 128 + hi == pos + n, cell))
                p0 = t * 128 + hi
            pos += n
        ntiles = (pos - s0 + 127) // 128
        chunk_ranges.append((tile_total, ntiles))
        tile_total += ntiles
    S_tiles = tile_total

    slabs = []
    meta_off = 0
    for c in range(CH):
        t0c, tc = chunk_ranges[c]
        s = t0c
        while s < t0c + tc:
            T = min(SLAB_T, t0c + tc - s)
            slabs.append({
                "chunk": c, "t0": s, "T": T, "meta_off": meta_off,
                "pieces": {t - s: pieces_by_tile.get(t, [])
                           for t in range(s, s + T)},
            })
            meta_off += 13 * T
            s += T
    struct = {
        "slabs": slabs,
        "S_tiles": S_tiles,
        "meta_cols": meta_off,
        "cell_slots": cell_slots,
        "cell_off": cell_off,
    }
    return struct, per_core_raw


def _per_core_arrays(struct, core_raw, edge_weights):
    """Packed per-slab metadata stream [128, meta_cols] int16."""
    o, d, eids, cnt = core_raw
    S_tiles = struct["S_tiles"]
    S = S_tiles * 128
    cell_off = struct["cell_off"]

    oid = np.full(S, PAD_OID, np.float32)
    kvi = np.zeros(S, np.int16)
    ew4 = np.zeros((S, H), np.float32)

    cell_edge_off = np.zeros(CH * NBLK + 1, np.int64)
    np.cumsum(cnt, out=cell_edge_off[1:])
    for cell in range(CH * NBLK):
        n = int(cnt[cell])
        if n == 0:
            continue
        e0 = cell_edge_off[cell]
        s0 = int(cell_off[cell])
        ch = cell // NBLK
        sl = slice(s0, s0 + n)
        el = slice(e0, e0 + n)
        oid[sl] = (o[el] & 127).astype(np.float32)
        kvi[sl] = (d[el] - CHUNK_BASE[ch]).astype(np.int16)
        ew4[sl] = edge_weights[eids[el]] * (HD ** -0.5)

    # slot-major -> tile layouts
    oid_t = np.ascontiguousarray(oid.reshape(S_tiles, 128).T).astype(BF16_NP)
    ew_t = np.ascontiguousarray(
        ew4.reshape(S_tiles, 128, H).transpose(1, 0, 2)).astype(BF16_NP)

    def wrap(run_vals):
        w = run_vals.reshape(-1, 16).T          # [16, len/16]
        return np.tile(w, (8, 1))               # [128, len/16]

    meta = np.zeros((128, struct["meta_cols"]), np.int16)
    for sl in struct["slabs"]:
        t0, T, mo = sl["t0"], sl["T"], sl["meta_off"]
        meta[:, mo:mo + 8 * T] = wrap(kvi[t0 * 128:(t0 + T) * 128])
        meta[:, mo + 8 * T:mo + 9 * T] = oid_t[:, t0:t0 + T].view(np.int16)
        meta[:, mo + 9 * T:mo + 13 * T] = (
            ew_t[:, t0:t0 + T, :].reshape(128, T * H).view(np.int16))
    return {"meta": meta}


def _build_graph(struct):
    nc = bacc.Bacc()
    slabs = struct["slabs"]

    xT = nc.declare_dram_parameter("xT", [D + 1, NT], BF16, isOutput=False)
    xTo = nc.declare_dram_parameter("xTo", [D + 1, NB], BF16, isOutput=False)
    wkv = nc.declare_dram_parameter("wkv", [D + 1, 2 * D], BF16, isOutput=False)
    wq = nc.declare_dram_parameter("wq", [D + 1, D], BF16, isOutput=False)
    wot = nc.declare_dram_parameter("wot", [D, D], F32, isOutput=False)
    boc = nc.declare_dram_parameter("boc", [128, D], F32, isOutput=False)
    gam = nc.declare_dram_parameter("gam", [128, D], F32, isOutput=False)
    iot = nc.declare_dram_parameter("iot", [128, 128, SLAB_T], BF16,
                                    isOutput=False)
    idn = nc.declare_dram_parameter("idn", [128, 128], BF16, isOutput=False)
    idnf = nc.declare_dram_parameter("idnf", [128, 128], F32, isOutput=False)
    xpb = nc.declare_dram_parameter("xpb", [NB, D], BF16, isOutput=False)
    met = nc.declare_dram_parameter("met", [128, struct["meta_cols"]], I16,
                                    isOutput=False)
    out = nc.declare_dram_parameter("out", [NB, D], F32, isOutput=True)

    kv_tab = nc.dram_tensor("kv_tab", [NT, 2 * D], BF16)

    with TileContext(nc) as tc:
        with tc.tile_pool(name="const", bufs=1) as cp:
            wkv_t = cp.tile([D + 1, 2 * D], BF16)
            nc.sync.dma_start(out=wkv_t[:], in_=wkv[:])
            wq_t = cp.tile([D + 1, D], BF16)
            nc.sync.dma_start(out=wq_t[:], in_=wq[:])
            wot_f = cp.tile([D, D], F32)
            nc.sync.dma_start(out=wot_f[:], in_=wot[:])
            wot_t = cp.tile([D, D], BF16)
            nc.vector.tensor_copy(wot_t[:], wot_f[:])
            boc_t = cp.tile([128, D], F32)
            nc.sync.dma_start(out=boc_t[:], in_=boc[:])
            gam_t = cp.tile([128, D], F32)
            nc.sync.dma_start(out=gam_t[:], in_=gam[:])
            iot_t = cp.tile([128, 128, SLAB_T], BF16)
            nc.sync.dma_start(out=iot_t[:], in_=iot[:])
            idn_t = cp.tile([128, 128], BF16)
            nc.sync.dma_start(out=idn_t[:], in_=idn[:])
            idnf_t = cp.tile([128, 128], F32)
            nc.sync.dma_start(out=idnf_t[:], in_=idnf[:])

            # q blocks for all own nodes, SBUF-resident [128, NBLK, D] bf16
            q_all = cp.tile([128, NBLK, D], BF16)
            # per-block f32 accumulators [128, NBLK, D+H]
            acc = cp.tile([128, NBLK, D + H], F32)
            nc.gpsimd.memset(acc[:], 0.0)

            # ---- q blocks: lhsT = xTo block cols, rhs = wq -> [128, 64]
            with (
                tc.tile_pool(name="qbx", bufs=2) as qbx,
                tc.tile_pool(name="qbp", bufs=2, space="PSUM") as qbp,
            ):
                for bat in range((NBLK + 7) // 8):
                    b0 = bat * 8
                    nb = min(8, NBLK - b0)
                    xs = qbx.tile([D + 1, 8 * 128], BF16, tag="xq")
                    nc.sync.dma_start(
                        out=xs[:, 0:nb * 128],
                        in_=xTo[:, b0 * 128:(b0 + nb) * 128])
                    ps = qbp.tile([128, 8, D], F32, tag="qp")
                    for j in range(nb):
                        nc.tensor.matmul(
                            out=ps[:, j, :],
                            lhsT=xs[:, j * 128:(j + 1) * 128],
                            rhs=wq_t[:],
                            start=True, stop=True)
                    nc.vector.tensor_copy(
                        q_all[:, b0:b0 + nb, :], ps[:, 0:nb, :])

            with (
                tc.tile_pool(name="p1x", bufs=2) as p1x,
                tc.tile_pool(name="p1s", bufs=2) as p1s,
                tc.tile_pool(name="p1p", bufs=2, space="PSUM") as p1p,
            ):
                # ---- kv table chunk builder: groups of 16 blocks (2048 rows)
                def build_kv_group(c, g):
                    # chunk 0 builds serially at startup while Act/SP are
                    # otherwise idle: alternate load/write across queues
                    ld = (nc.gpsimd if g % 2 == 0 else nc.sync) \
                        if c == 0 else nc.sync
                    wr = (nc.sync if g % 2 == 0 else nc.gpsimd) \
                        if c == 0 else nc.sync
                    r0 = CHUNK_BASE[c] + g * 2048
                    nrow = min(2048, CHUNK_ROWS[c] - g * 2048)
                    nblk4 = (nrow + 511) // 512     # 4-block psum batches
                    xs = p1x.tile([D + 1, 2048], BF16, tag="xs")
                    ld.dma_start(
                        out=xs[:, 0:nrow], in_=xT[:, r0:r0 + nrow])
                    sb = p1s.tile([128, 16, 2 * D], BF16, tag="sb")
                    for bat in range(nblk4):
                        ps = p1p.tile([128, 4, 2 * D], F32, tag="kp")
                        for j in range(4):
                            nc.tensor.matmul(
                                out=ps[:, j, :],
                                lhsT=xs[:, bat * 512 + j * 128:
                                        bat * 512 + (j + 1) * 128],
                                rhs=wkv_t[:],
                                start=True, stop=True)
                        if c == 0 and bat % 2 == 0:
                            nc.vector.tensor_copy(
                                sb[:, bat * 4:(bat + 1) * 4, :]
                                    .rearrange("p a d -> p (a d)"),
                                ps[:].rearrange("p a d -> p (a d)"))
                        else:
                            nc.scalar.copy(
                                sb[:, bat * 4:(bat + 1) * 4, :]
                                    .rearrange("p a d -> p (a d)"),
                                ps[:].rearrange("p a d -> p (a d)"))
                    wr.dma_start(
                        out=kv_tab[r0:r0 + nrow, :]
                            .rearrange("(a p) d -> p a d", p=128),
                        in_=sb[:, 0:nrow // 128, :])

                # ---- edge-processing slab
                with (
                    tc.tile_pool(name="gat", bufs=3) as gp,
                    tc.tile_pool(name="mp", bufs=3) as mp,
                    tc.tile_pool(name="sp", bufs=3) as sp,
                    tc.tile_pool(name="qs", bufs=3) as qp,
                    tc.tile_pool(name="wk", bufs=2) as wp,
                    tc.tile_pool(name="stp", bufs=2, space="PSUM") as stp,
                    tc.tile_pool(name="qpp", bufs=2, space="PSUM") as qpp,
                    tc.tile_pool(name="bpp", bufs=2, space="PSUM") as bpp,
                ):
                    state = {}
                    live = {}

                    def slab_A(sl):
                        c, t0, T = sl["chunk"], sl["t0"], sl["T"]
                        mo = sl["meta_off"]
                        mt = mp.tile([128, 13 * SLAB_T], I16, tag="mt")
                        nc.sync.dma_start(
                            out=mt[:, 0:13 * T], in_=met[:, mo:mo + 13 * T])
                        kvi_v = mt[:, 0:8 * T]
                        kvg = gp.tile([128, SLAB_T, 2 * D], BF16, tag="kvg")
                        nc.gpsimd.dma_gather(
                            out_ap=kvg[:, 0:T, :],
                            in_ap=kv_tab[CHUNK_BASE[c]:
                                          CHUNK_BASE[c] + CHUNK_ROWS[c], :],
                            idxs_ap=kvi_v,
                            num_idxs=T * 128,
                            num_idxs_reg=T * 128,
                            elem_size=2 * D,
                            single_packet=False)
                        oid_v = mt[:, 8 * T:9 * T].bitcast(BF16)
                        sel = sp.tile([128, 128, SLAB_T], BF16, tag="sel")
                        nc.vector.tensor_tensor(
                            out=sel[:, :, 0:T],
                            in0=oid_v.rearrange("p (o t) -> p o t", o=1)
                                .to_broadcast([128, 128, T]),
                            in1=iot_t[:, :, 0:T],
                            op=mybir.AluOpType.is_equal)
                        state[sl["t0"]] = (mt, kvg, sel)

                    def slab_B(sl):
                        c, t0, T = sl["chunk"], sl["t0"], sl["T"]
                        mt, kvg, sel = state[t0]

                        # q broadcast: transpose sel -> selT, q = selT^T @ Q,
                        # qk = q*k straight from PSUM (gpsimd can't read PSUM)
                        qk = qp.tile([128, SLAB_T, D], BF16, tag="qk")
                        nbat = (T + QB - 1) // QB
                        for b in range(nbat):
                            i0 = b * QB
                            nb_ = min(QB, T - i0)
                            tp = stp.tile([128, QB, 128], BF16, tag="tp")
                            for j in range(nb_):
                                nc.tensor.transpose(
                                    out=tp[:, j, :],
                                    in_=sel[:, :, i0 + j],
                                    identity=idn_t[:])
                            st = sp.tile([128, QB, 128], BF16, tag="st")
                            nc.scalar.copy(
                                st[:, 0:nb_, :].rearrange("p a d -> p (a d)"),
                                tp[:, 0:nb_, :].rearrange("p a d -> p (a d)"))
                            qps = qpp.tile([128, QB, D], F32, tag="qps")
                            for j in range(nb_):
                                for (blk, lo, hi, _f, _l, _c) in \
                                        sl["pieces"][i0 + j]:
                                    nc.tensor.matmul(
                                        out=qps[lo:hi, j, :],
                                        lhsT=st[:, j, lo:hi],
                                        rhs=q_all[:, blk, :],
                                        start=True, stop=True)
                            nc.vector.tensor_tensor(
                                out=qk[:, i0:i0 + nb_, :],
                                in0=qps[:, 0:nb_, :],
                                in1=kvg[:, i0:i0 + nb_, 0:D],
                                op=mybir.AluOpType.mult)
                        state[t0] = (mt, kvg, sel, qk)

                    def slab_C(sl):
                        c, t0, T = sl["chunk"], sl["t0"], sl["T"]
                        mt, kvg, sel, qk = state.pop(t0)
                        ew_v = mt[:, 9 * T:13 * T].bitcast(BF16) \
                            .rearrange("p (t h) -> p t h", h=H)

                        t1 = wp.tile([128, SLAB_T, H, HD // 2], BF16, tag="t1")
                        qk4 = qk.rearrange("p t (h d) -> p t h d", h=H)
                        nc.gpsimd.tensor_tensor(
                            out=t1[:, 0:T, :, :],
                            in0=qk4[:, 0:T, :, 0:HD // 2],
                            in1=qk4[:, 0:T, :, HD // 2:HD],
                            op=mybir.AluOpType.add)
                        t2 = wp.tile([128, SLAB_T, H, HD // 4], BF16, tag="t2")
                        nc.vector.tensor_tensor(
                            out=t2[:, 0:T, :, :],
                            in0=t1[:, 0:T, :, 0:HD // 4],
                            in1=t1[:, 0:T, :, HD // 4:HD // 2],
                            op=mybir.AluOpType.add)
                        t3 = wp.tile([128, SLAB_T, H, HD // 8], BF16, tag="t3")
                        nc.vector.tensor_tensor(
                            out=t3[:, 0:T, :, :],
                            in0=t2[:, 0:T, :, 0:HD // 8],
                            in1=t2[:, 0:T, :, HD // 8:HD // 4],
                            op=mybir.AluOpType.add)
                        sc = wp.tile([128, SLAB_T, H], F32, tag="sc")
                        nc.vector.tensor_reduce(
                            out=sc[:, 0:T, :], in_=t3[:, 0:T, :, :],
                            axis=mybir.AxisListType.X, op=mybir.AluOpType.add)
                        ws = wp.tile([128, SLAB_T, H], F32, tag="ws")
                        nc.gpsimd.tensor_tensor(
                            out=ws[:, 0:T, :], in0=sc[:, 0:T, :],
                            in1=ew_v, op=mybir.AluOpType.mult)
                        ex = wp.tile([128, SLAB_T, H], BF16, tag="ex")
                        nc.scalar.activation(
                            out=ex[:, 0:T, :], in_=ws[:, 0:T, :],
                            func=mybir.ActivationFunctionType.Exp)

                        ctb = wp.tile([128, SLAB_T, D + H], BF16, tag="ctb")
                        nc.vector.tensor_copy(
                            ctb[:, 0:T, D:D + H], ex[:, 0:T, :])
                        nc.gpsimd.tensor_tensor(
                            out=ctb[:, 0:T, 0:D]
                                .rearrange("p t (e h) -> p t e h", h=H),
                            in0=kvg[:, 0:T, D:2 * D]
                                .rearrange("p t (e h) -> p t e h", h=H),
                            in1=ex[:, 0:T, :]
                                .rearrange("p t (o h) -> p t o h", o=1)
                                .to_broadcast([128, T, HD, H]),
                            op=mybir.AluOpType.mult)

                        # segment-sum into per-cell psum slices; flush 4
                        # consecutive blocks per DVE add (amortize PSUM init)
                        for i in range(T):
                            for (blk, lo, hi, first, last, cell) in \
                                    sl["pieces"][i]:
                                grp = blk // 4
                                key = (c, grp)
                                if key not in live:
                                    live[key] = bpp.tile(
                                        [128, 4, D + H], F32, tag="bps",
                                        name=f"bps{c}_{grp}")
                                bps = live[key]
                                nc.tensor.matmul(
                                    out=bps[:, blk % 4, :],
                                    lhsT=sel[lo:hi, :, i],
                                    rhs=ctb[lo:hi, i, :],
                                    start=first, stop=last)
                                if last and blk == min(4 * grp + 3, NBLK - 1):
                                    nb_g = blk - 4 * grp + 1
                                    nc.vector.tensor_tensor(
                                        out=acc[:, 4 * grp:blk + 1, :],
                                        in0=acc[:, 4 * grp:blk + 1, :],
                                        in1=bps[:, 0:nb_g, :],
                                        op=mybir.AluOpType.add)
                                    del live[key]

                    # ---- pipeline: build chunk 0, then A/B/C software
                    # pipeline over slabs with chunk c+1 builds interleaved
                    NGRP = [(r + 2047) // 2048 for r in CHUNK_ROWS]
                    for g in range(NGRP[0]):
                        build_kv_group(0, g)
                    ns = len(slabs)
                    # build-group schedule: spread chunk c+1's groups
                    # over chunk c's A-stage iterations
                    binj = {}
                    for c in range(CH - 1):
                        idxs = [i for i, s in enumerate(slabs)
                                if s["chunk"] == c]
                        ng = NGRP[c + 1]
                        for j, g in enumerate(range(ng)):
                            at = idxs[min(j * len(idxs) // ng,
                                          len(idxs) - 1)]
                            binj.setdefault(at, []).append((c + 1, g))
                    slab_A(slabs[0])
                    for k in range(ns + 2):
                        if k - 2 >= 0:
                            slab_C(slabs[k - 2])
                        if k < ns:
                            for (bc, bg) in binj.get(k, []):
                                build_kv_group(bc, bg)
                        if k + 1 < ns:
                            slab_A(slabs[k + 1])
                        if 0 <= k - 1 < ns:
                            slab_B(slabs[k - 1])

                # ---- epilogue over all blocks
                with (
                    tc.tile_pool(name="ep", bufs=3) as ep,
                    tc.tile_pool(name="ett", bufs=2, space="PSUM") as et,
                    tc.tile_pool(name="eo", bufs=2, space="PSUM") as eo,
                ):
                    EB = 14                       # blocks per epilogue slab
                    for s0 in range(0, NBLK, EB):
                        nb = min(EB, NBLK - s0)
                        zr = ep.tile([128, EB, H], F32, tag="zr")
                        nc.vector.tensor_scalar_add(
                            zr[:, 0:nb, :], acc[:, s0:s0 + nb, D:D + H],
                            1e-16)
                        nc.vector.reciprocal(zr[:, 0:nb, :], zr[:, 0:nb, :])
                        vals = ep.tile([128, EB, D], F32, tag="vals")
                        nc.gpsimd.tensor_tensor(
                            out=vals[:, 0:nb, :]
                                .rearrange("p b (e h) -> p b e h", h=H),
                            in0=acc[:, s0:s0 + nb, 0:D]
                                .rearrange("p b (e h) -> p b e h", h=H),
                            in1=zr[:, 0:nb, :]
                                .rearrange("p b (o h) -> p b o h", o=1)
                                .to_broadcast([128, nb, HD, H]),
                            op=mybir.AluOpType.mult)
                        po = eo.tile([128, EB, D], F32, tag="po")
                        for i in range(nb):
                            pt = et.tile([D, 128], F32, tag="pt")
                            nc.tensor.transpose(
                                out=pt[:], in_=vals[:, i, :],
                                identity=idnf_t[:])
                            vT = ep.tile([D, 128], BF16, tag="vT")
                            nc.scalar.copy(vT[:], pt[:])
                            nc.tensor.matmul(
                                out=po[:, i, :], lhsT=vT[:], rhs=wot_t[:],
                                start=True, stop=True)
                        nmu = ep.tile([128, EB], F32, tag="nmu")
                        nc.vector.tensor_reduce(
                            out=nmu[:, 0:nb], in_=po[:, 0:nb, :],
                            axis=mybir.AxisListType.X, op=mybir.AluOpType.add)
                        nc.vector.tensor_scalar_mul(
                            nmu[:, 0:nb], nmu[:, 0:nb], -1.0 / D)
                        ct = ep.tile([128, EB, D], F32, tag="ct")
                        nc.vector.tensor_tensor(
                            out=ct[:, 0:nb, :], in0=po[:, 0:nb, :],
                            in1=nmu[:, 0:nb].rearrange("p (b o) -> p b o", o=1)
                                .to_broadcast([128, nb, D]),
                            op=mybir.AluOpType.add)
                        nc.gpsimd.tensor_tensor(
                            out=ct[:, 0:nb, :], in0=ct[:, 0:nb, :],
                            in1=boc_t[:].rearrange("p (o d) -> p o d", o=1)
                                .to_broadcast([128, nb, D]),
                            op=mybir.AluOpType.add)
                        sq = ep.tile([128, EB, D], F32, tag="sq")
                        nc.gpsimd.tensor_tensor(
                            out=sq[:, 0:nb, :], in0=ct[:, 0:nb, :],
                            in1=ct[:, 0:nb, :], op=mybir.AluOpType.mult)
                        v1 = ep.tile([128, EB], F32, tag="v1")
                        nc.vector.tensor_reduce(
                            out=v1[:, 0:nb], in_=sq[:, 0:nb, :],
                            axis=mybir.AxisListType.X, op=mybir.AluOpType.add)
                        nc.vector.tensor_scalar(
                            out=v1[:, 0:nb], in0=v1[:, 0:nb],
                            scalar1=1.0 / D, scalar2=LN_EPS,
                            op0=mybir.AluOpType.mult,
                            op1=mybir.AluOpType.add)
                        nc.vector.reciprocal(v1[:, 0:nb], v1[:, 0:nb])
                        rstd = ep.tile([128, EB], F32, tag="rstd")
                        nc.scalar.sqrt(rstd[:, 0:nb], v1[:, 0:nb])
                        xb = ep.tile([128, EB, D], BF16, tag="xb")
                        nc.sync.dma_start(
                            out=xb[:, 0:nb, :],
                            in_=xpb[s0 * 128:(s0 + nb) * 128, :]
                                .rearrange("(a p) d -> p a d", p=128))
                        ot = ep.tile([128, EB, D], F32, tag="ot")
                        nc.gpsimd.tensor_tensor(
                            out=ot[:, 0:nb, :], in0=ct[:, 0:nb, :],
                            in1=rstd[:, 0:nb]
                                .rearrange("p (b o) -> p b o", o=1)
                                .to_broadcast([128, nb, D]),
                            op=mybir.AluOpType.mult)
                        nc.gpsimd.tensor_tensor(
                            out=ot[:, 0:nb, :], in0=ot[:, 0:nb, :],
                            in1=gam_t[:].rearrange("p (o d) -> p o d", o=1)
                                .to_broadcast([128, nb, D]),
                            op=mybir.AluOpType.mult)
                        nc.gpsimd.tensor_tensor(
                            out=ot[:, 0:nb, :], in0=ot[:, 0:nb, :],
                            in1=xb[:, 0:nb, :], op=mybir.AluOpType.add)
                        nc.sync.dma_start(
                            out=out[s0 * 128:(s0 + nb) * 128, :]
                                .rearrange("(a p) d -> p a d", p=128),
                            in_=ot[:, 0:nb, :])
    return nc


def kernel(x, edge_index, edge_weights, Wq, bq, Wk, bk, Wv, bv, Wo, bo,
           gamma, beta):
    x = np.asarray(x, np.float32)
    edge_index = np.asarray(edge_index)
    edge_weights = np.asarray(edge_weights, np.float32)
    origins = np.asarray(edge_index[0], np.int64)
    dests = np.asarray(edge_index[1], np.int64)

    struct, per_core_raw = _build_structure(origins, dests)
    nc = _build_graph(struct)
    nc.finalize()

    xT = np.zeros((D + 1, NT), np.float32)
    xT[:D, :N] = x.T
    xT[D] = 1.0
    xT = xT.astype(BF16_NP)
    vperm = (np.arange(H)[None, :] * HD + np.arange(HD)[:, None]).ravel()
    wkv = np.zeros((D + 1, 2 * D), np.float32)
    wkv[:D, :D] = np.asarray(Wk, np.float32).T
    wkv[:D, D:] = np.asarray(Wv, np.float32).T[:, vperm]
    wkv[D, :D] = np.asarray(bk, np.float32)
    wkv[D, D:] = np.asarray(bv, np.float32)[vperm]
    wq = np.zeros((D + 1, D), np.float32)
    wq[:D, :] = np.asarray(Wq, np.float32).T
    wq[D, :] = np.asarray(bq, np.float32)
    wkv = wkv.astype(BF16_NP)
    wq = wq.astype(BF16_NP)
    wot = np.ascontiguousarray(np.asarray(Wo, np.float32).T[vperm, :])
    bo_ = np.asarray(bo, np.float32)
    boc = np.tile((bo_ - bo_.mean())[None, :], (128, 1)).astype(np.float32)
    gam_t = np.tile(np.asarray(gamma, np.float32)[None, :], (128, 1))
    iot = np.tile(np.arange(128, dtype=np.float32)[None, :, None],
                  (128, 1, SLAB_T)).astype(BF16_NP)
    idn = np.eye(128, dtype=np.float32)

    in_maps = []
    for c in range(NCORES):
        data = _per_core_arrays(struct, per_core_raw[c], edge_weights)
        xTo = np.zeros((D + 1, NB), np.float32)
        xTo[:D, :NOWN] = x[c * NOWN:(c + 1) * NOWN].T
        xTo[D] = 1.0
        xTo = xTo.astype(BF16_NP)
        xpb = np.zeros((NB, D), np.float32)
        xpb[:NOWN] = x[c * NOWN:(c + 1) * NOWN] + np.asarray(beta, np.float32)
        in_maps.append({
            "xT": xT, "xTo": xTo, "wkv": wkv, "wq": wq, "wot": wot,
            "boc": boc, "gam": gam_t, "iot": iot,
            "idn": idn.astype(BF16_NP), "idnf": idn,
            "xpb": xpb.astype(BF16_NP), "met": data["meta"],
        })

    global LAST_SIM_NS
    if SIMULATE_COST:
        from concourse import bass_interp
        sim = bass_interp.CoreSim(nc, no_exec=True, publish_trace=False)
        sim.event_loop()
        LAST_SIM_NS = int(sim.time)

    res = run_bass_kernel_spmd(nc, in_maps, core_ids=list(range(NCORES)),
                               trace=TRACE)
    global LAST_RESULT
    LAST_RESULT = res
    outs = [np.asarray(res.results[i]["out"])[:NOWN] for i in range(NCORES)]
    return np.concatenate(outs, axis=0).astype(np.float32)


TRACE = False
SIMULATE_COST = False
LAST_RESULT = None
LAST_SIM_NS = None


# revision 6
# speedup vs baseline: 1.4473x; 1.0245x over previous
"""Trainium2 Bass kernel v2 for BaseDependentAttentionLayer (GNN message passing).

Design vs v1 baseline (739us cost-model):
  - Chunk-outer pipeline: kv-table chunk c+1 builds (PE/DMA) while chunk c's
    edges process (DVE/Pool/Act).  Removes the 175us serial phase-1 stall.
  - No q gather: per tile, PE transposes the sel one-hot (slot x origin),
    Act copies selT to SBUF, PE matmul broadcasts the 128-node q block to
    slots.  Kills 250k gather descs of DMA + Pool desc-gen.
  - Per-block accumulators live in SBUF f32 across all 4 chunks; single
    final epilogue (one act-table switch, big-slab instructions).
  - Metadata packed per slab in one i16 stream (kvi | oid | ew-bf16).
  - Engine placement: sel/qk/tree/reduce/ctb on DVE; ws + psum flushes +
    phase1 copies on Pool; selT copies + exp on Act; DMA issues on SP.
"""

import sys

sys.path.insert(0, "/opt/trn_rl_repo")

import numpy as np
import ml_dtypes

import concourse.bass as bass
import concourse.bacc as bacc
import concourse.mybir as mybir
from concourse.tile import TileContext
from concourse.bass_utils import run_bass_kernel_spmd

N = 100000
E = 1600000
D = 64
H = 4
HD = 16
NCORES = 8
NOWN = 12500            # nodes owned per core
NBLK = 98               # 128-node blocks per core
NB = NBLK * 128         # 12544 padded own nodes
NT = 100352             # padded global table rows (= 4 * 25088)
CH = 4                  # dest chunks (int16 gather index limit)
# uneven chunks: tiny chunk 0 builds fast so edge processing starts early;
# the big chunks build in the shadow of processing (all rows <= 32767)
CHUNK_BASE = [0, 16384, 45056, 72704]
CHUNK_ROWS = [16384, 28672, 27648, 27648]
SLAB_T = 48             # edge tiles per processing slab
QB = 8                  # tiles per selT/q-broadcast batch
LN_EPS = 1e-5
PAD_OID = 200.0

F32 = mybir.dt.float32
BF16 = mybir.dt.bfloat16
I16 = mybir.dt.int16
BF16_NP = ml_dtypes.bfloat16


def _build_structure(origins, dests):
    """Chunk-major packed cell layout (no per-cell 128-rounding).

    Cells (chunk, block) are packed back-to-back; only chunk streams are
    padded to 128.  Tiles spanning cell boundaries are handled with
    partition-sliced matmuls ("pieces").  Structure is shared by cores:
    per-cell slot count = max over cores.
    """
    owner = origins // NOWN
    per_core_raw = []
    cnts = np.zeros((NCORES, CH * NBLK), np.int64)
    for c in range(NCORES):
        m = owner == c
        o = (origins[m] - c * NOWN).astype(np.int32)
        d = dests[m].astype(np.int32)
        eids = np.nonzero(m)[0]
        blk = o >> 7
        chunk = np.searchsorted(np.asarray(CHUNK_BASE), d, side="right") - 1
        cell = chunk * NBLK + blk          # chunk-major
        order = np.argsort(cell, kind="stable")
        o, d, eids = o[order], d[order], eids[order]
        cnt = np.bincount(cell[order], minlength=CH * NBLK)
        cnts[c] = cnt
        per_core_raw.append((o, d, eids, cnt))
    # 128-aligned cells: base-64 matmul pieces compile but fail at runtime
    cell_slots = ((cnts.max(0) + 127) // 128) * 128

    cell_off = np.zeros(CH * NBLK, np.int64)
    pieces_by_tile = {}
    tile_total = 0
    chunk_ranges = []                      # (tile0, ntiles) per chunk
    for c in range(CH):
        s0 = tile_total * 128
        pos = s0
        for b in range(NBLK):
            cell = c * NBLK + b
            cell_off[cell] = pos
            n = int(cell_slots[cell])
            if n == 0:
                continue
            p0 = pos
            while p0 < pos + n:
                t = p0 // 128
                lo = p0 - t * 128
                hi = min(128, pos + n - t * 128)
                pieces_by_tile.setdefault(t, []).append(
                    (b, lo, hi, p0 == pos, t * 128 + hi == pos + n, cell))
                p0 = t * 128 + hi
            pos += n
        ntiles = (pos - s0 + 127) // 128
        chunk_ranges.append((tile_total, ntiles))
        tile_total += ntiles
    S_tiles = tile_total

    slabs = []
    meta_off = 0
    for c in range(CH):
        t0c, tc = chunk_ranges[c]
        s = t0c
        while s < t0c + tc:
            T = min(SLAB_T, t0c + tc - s)
            slabs.append({
                "chunk": c, "t0": s, "T": T, "meta_off": meta_off,
                "pieces": {t - s: pieces_by_tile.get(t, [])
                           for t in range(s, s + T)},
            })
            meta_off += 13 * T
            s += T
    struct = {
        "slabs": slabs,
        "S_tiles": S_tiles,
        "meta_cols": meta_off,
        "cell_slots": cell_slots,
        "cell_off": cell_off,
    }
    return struct, per_core_raw


def _per_core_arrays(struct, core_raw, edge_weights):
    """Packed per-slab metadata stream [128, meta_cols] int16."""
    o, d, eids, cnt = core_raw
    S_tiles = struct["S_tiles"]
    S = S_tiles * 128
    cell_off = struct["cell_off"]

    oid = np.full(S, PAD_OID, np.float32)
    kvi = np.zeros(S, np.int16)
    ew4 = np.zeros((S, H), np.float32)

    cell_edge_off = np.zeros(CH * NBLK + 1, np.int64)
    np.cumsum(cnt, out=cell_edge_off[1:])
    for cell in range(CH * NBLK):
        n = int(cnt[cell])
        if n == 0:
            continue
        e0 = cell_edge_off[cell]
        s0 = int(cell_off[cell])
        ch = cell // NBLK
        sl = slice(s0, s0 + n)
        el = slice(e0, e0 + n)
        oid[sl] = (o[el] & 127).astype(np.float32)
        kvi[sl] = (d[el] - CHUNK_BASE[ch]).astype(np.int16)
        ew4[sl] = edge_weights[eids[el]] * (HD ** -0.5)

    # slot-major -> tile layouts
    oid_t = np.ascontiguousarray(oid.reshape(S_tiles, 128).T).astype(BF16_NP)
    ew_t = np.ascontiguousarray(
        ew4.reshape(S_tiles, 128, H).transpose(1, 0, 2)).astype(BF16_NP)

    def wrap(run_vals):
        w = run_vals.reshape(-1, 16).T          # [16, len/16]
        return np.tile(w, (8, 1))               # [128, len/16]

    meta = np.zeros((128, struct["meta_cols"]), np.int16)
    for sl in struct["slabs"]:
        t0, T, mo = sl["t0"], sl["T"], sl["meta_off"]
        meta[:, mo:mo + 8 * T] = wrap(kvi[t0 * 128:(t0 + T) * 128])
        meta[:, mo + 8 * T:mo + 9 * T] = oid_t[:, t0:t0 + T].view(np.int16)
        meta[:, mo + 9 * T:mo + 13 * T] = (
            ew_t[:, t0:t0 + T, :].reshape(128, T * H).view(np.int16))
    return {"meta": meta}


def _build_graph(struct):
    nc = bacc.Bacc()
    slabs = struct["slabs"]

    xT = nc.declare_dram_parameter("xT", [D + 1, NT], BF16, isOutput=False)
    xTo = nc.declare_dram_parameter("xTo", [D + 1, NB], BF16, isOutput=False)
    wkv = nc.declare_dram_parameter("wkv", [D + 1, 2 * D], BF16, isOutput=False)
    wq = nc.declare_dram_parameter("wq", [D + 1, D], BF16, isOutput=False)
    wot = nc.declare_dram_parameter("wot", [D, D], F32, isOutput=False)
    boc = nc.declare_dram_parameter("boc", [128, D], F32, isOutput=False)
    gam = nc.declare_dram_parameter("gam", [128, D], F32, isOutput=False)
    iot = nc.declare_dram_parameter("iot", [128, 128, SLAB_T], BF16,
                                    isOutput=False)
    idn = nc.declare_dram_parameter("idn", [128, 128], BF16, isOutput=False)
    idnf = nc.declare_dram_parameter("idnf", [128, 128], F32, isOutput=False)
    xpb = nc.declare_dram_parameter("xpb", [NB, D], BF16, isOutput=False)
    met = nc.declare_dram_parameter("met", [128, struct["meta_cols"]], I16,
                                    isOutput=False)
    out = nc.declare_dram_parameter("out", [NB, D], F32, isOutput=True)

    kv_tab = nc.dram_tensor("kv_tab", [NT, 2 * D], BF16)

    with TileContext(nc) as tc:
        with tc.tile_pool(name="const", bufs=1) as cp:
            wkv_t = cp.tile([D + 1, 2 * D], BF16)
            nc.sync.dma_start(out=wkv_t[:], in_=wkv[:])
            wq_t = cp.tile([D + 1, D], BF16)
            nc.sync.dma_start(out=wq_t[:], in_=wq[:])
            wot_f = cp.tile([D, D], F32)
            nc.sync.dma_start(out=wot_f[:], in_=wot[:])
            wot_t = cp.tile([D, D], BF16)
            nc.vector.tensor_copy(wot_t[:], wot_f[:])
            boc_t = cp.tile([128, D], F32)
            nc.sync.dma_start(out=boc_t[:], in_=boc[:])
            gam_t = cp.tile([128, D], F32)
            nc.sync.dma_start(out=gam_t[:], in_=gam[:])
            iot_t = cp.tile([128, 128, SLAB_T], BF16)
            nc.sync.dma_start(out=iot_t[:], in_=iot[:])
            idn_t = cp.tile([128, 128], BF16)
            nc.sync.dma_start(out=idn_t[:], in_=idn[:])
            idnf_t = cp.tile([128, 128], F32)
            nc.sync.dma_start(out=idnf_t[:], in_=idnf[:])

            # q blocks for all own nodes, SBUF-resident [128, NBLK, D] bf16
            q_all = cp.tile([128, NBLK, D], BF16)
            # per-block f32 accumulators [128, NBLK, D+H]
            acc = cp.tile([128, NBLK, D + H], F32)
            nc.gpsimd.memset(acc[:], 0.0)

            # ---- q blocks: lhsT = xTo block cols, rhs = wq -> [128, 64]
            with (
                tc.tile_pool(name="qbx", bufs=2) as qbx,
                tc.tile_pool(name="qbp", bufs=2, space="PSUM") as qbp,
            ):
                for bat in range((NBLK + 7) // 8):
                    b0 = bat * 8
                    nb = min(8, NBLK - b0)
                    xs = qbx.tile([D + 1, 8 * 128], BF16, tag="xq")
                    nc.scalar.dma_start(
                        out=xs[:, 0:nb * 128],
                        in_=xTo[:, b0 * 128:(b0 + nb) * 128])
                    ps = qbp.tile([128, 8, D], F32, tag="qp")
                    for j in range(nb):
                        nc.tensor.matmul(
                            out=ps[:, j, :],
                            lhsT=xs[:, j * 128:(j + 1) * 128],
                            rhs=wq_t[:],
                            start=True, stop=True)
                    nc.vector.tensor_copy(
                        q_all[:, b0:b0 + nb, :], ps[:, 0:nb, :])

            with (
                tc.tile_pool(name="p1x", bufs=2) as p1x,
                tc.tile_pool(name="p1s", bufs=2) as p1s,
                tc.tile_pool(name="p1p", bufs=2, space="PSUM") as p1p,
            ):
                # ---- kv table chunk builder: groups of 16 blocks (2048 rows)
                def build_kv_group(c, g):
                    # chunk 0 builds serially at startup while Act/SP are
                    # otherwise idle: alternate load/write across queues
                    ld = (nc.gpsimd if g % 2 == 0 else nc.sync) \
                        if c == 0 else nc.sync
                    wr = (nc.sync if g % 2 == 0 else nc.gpsimd) \
                        if c == 0 else nc.sync
                    r0 = CHUNK_BASE[c] + g * 2048
                    nrow = min(2048, CHUNK_ROWS[c] - g * 2048)
                    nblk4 = (nrow + 511) // 512     # 4-block psum batches
                    xs = p1x.tile([D + 1, 2048], BF16, tag="xs")
                    ld.dma_start(
                        out=xs[:, 0:nrow], in_=xT[:, r0:r0 + nrow])
                    sb = p1s.tile([128, 16, 2 * D], BF16, tag="sb")
                    for bat in range(nblk4):
                        ps = p1p.tile([128, 4, 2 * D], F32, tag="kp")
                        for j in range(4):
                            nc.tensor.matmul(
                                out=ps[:, j, :],
                                lhsT=xs[:, bat * 512 + j * 128:
                                        bat * 512 + (j + 1) * 128],
                                rhs=wkv_t[:],
                                start=True, stop=True)
                        if c == 0 and bat % 2 == 0:
                            nc.vector.tensor_copy(
                                sb[:, bat * 4:(bat + 1) * 4, :]
                                    .rearrange("p a d -> p (a d)"),
                                ps[:].rearrange("p a d -> p (a d)"))
                        else:
                            nc.scalar.copy(
                                sb[:, bat * 4:(bat + 1) * 4, :]
                                    .rearrange("p a d -> p (a d)"),
                                ps[:].rearrange("p a d -> p (a d)"))
                    wr.dma_start(
                        out=kv_tab[r0:r0 + nrow, :]
                            .rearrange("(a p) d -> p a d", p=128),
                        in_=sb[:, 0:nrow // 128, :])

                # ---- edge-processing slab
                with (
                    tc.tile_pool(name="gat", bufs=3) as gp,
                    tc.tile_pool(name="mp", bufs=3) as mp,
                    tc.tile_pool(name="sp", bufs=3) as sp,
                    tc.tile_pool(name="qs", bufs=3) as qp,
                    tc.tile_pool(name="wk", bufs=2) as wp,
                    tc.tile_pool(name="stp", bufs=2, space="PSUM") as stp,
                    tc.tile_pool(name="qpp", bufs=2, space="PSUM") as qpp,
                    tc.tile_pool(name="bpp", bufs=2, space="PSUM") as bpp,
                ):
                    state = {}
                    live = {}

                    def slab_A(sl):
                        c, t0, T = sl["chunk"], sl["t0"], sl["T"]
                        mo = sl["meta_off"]
                        mt = mp.tile([128, 13 * SLAB_T], I16, tag="mt")
                        nc.sync.dma_start(
                            out=mt[:, 0:13 * T], in_=met[:, mo:mo + 13 * T])
                        kvi_v = mt[:, 0:8 * T]
                        kvg = gp.tile([128, SLAB_T, 2 * D], BF16, tag="kvg")
                        nc.gpsimd.dma_gather(
                            out_ap=kvg[:, 0:T, :],
                            in_ap=kv_tab[CHUNK_BASE[c]:
                                          CHUNK_BASE[c] + CHUNK_ROWS[c], :],
                            idxs_ap=kvi_v,
                            num_idxs=T * 128,
                            num_idxs_reg=T * 128,
                            elem_size=2 * D,
                            single_packet=False)
                        oid_v = mt[:, 8 * T:9 * T].bitcast(BF16)
                        sel = sp.tile([128, 128, SLAB_T], BF16, tag="sel")
                        nc.vector.tensor_tensor(
                            out=sel[:, :, 0:T],
                            in0=oid_v.rearrange("p (o t) -> p o t", o=1)
                                .to_broadcast([128, 128, T]),
                            in1=iot_t[:, :, 0:T],
                            op=mybir.AluOpType.is_equal)
                        state[sl["t0"]] = (mt, kvg, sel)

                    def slab_B(sl):
                        c, t0, T = sl["chunk"], sl["t0"], sl["T"]
                        mt, kvg, sel = state[t0]

                        # q broadcast: transpose sel -> selT, q = selT^T @ Q,
                        # qk = q*k straight from PSUM (gpsimd can't read PSUM)
                        qk = qp.tile([128, SLAB_T, D], BF16, tag="qk")
                        nbat = (T + QB - 1) // QB
                        for b in range(nbat):
                            i0 = b * QB
                            nb_ = min(QB, T - i0)
                            tp = stp.tile([128, QB, 128], BF16, tag="tp")
                            for j in range(nb_):
                                nc.tensor.transpose(
                                    out=tp[:, j, :],
                                    in_=sel[:, :, i0 + j],
                                    identity=idn_t[:])
                            st = sp.tile([128, QB, 128], BF16, tag="st")
                            nc.scalar.copy(
                                st[:, 0:nb_, :].rearrange("p a d -> p (a d)"),
                                tp[:, 0:nb_, :].rearrange("p a d -> p (a d)"))
                            qps = qpp.tile([128, QB, D], F32, tag="qps")
                            for j in range(nb_):
                                for (blk, lo, hi, _f, _l, _c) in \
                                        sl["pieces"][i0 + j]:
                                    nc.tensor.matmul(
                                        out=qps[lo:hi, j, :],
                                        lhsT=st[:, j, lo:hi],
                                        rhs=q_all[:, blk, :],
                                        start=True, stop=True)
                            nc.vector.tensor_tensor(
                                out=qk[:, i0:i0 + nb_, :],
                                in0=qps[:, 0:nb_, :],
                                in1=kvg[:, i0:i0 + nb_, 0:D],
                                op=mybir.AluOpType.mult)
                        state[t0] = (mt, kvg, sel, qk)

                    def slab_C(sl):
                        c, t0, T = sl["chunk"], sl["t0"], sl["T"]
                        mt, kvg, sel, qk = state.pop(t0)
                        ew_v = mt[:, 9 * T:13 * T].bitcast(BF16) \
                            .rearrange("p (t h) -> p t h", h=H)

                        t1 = wp.tile([128, SLAB_T, H, HD // 2], BF16, tag="t1")
                        qk4 = qk.rearrange("p t (h d) -> p t h d", h=H)
                        nc.gpsimd.tensor_tensor(
                            out=t1[:, 0:T, :, :],
                            in0=qk4[:, 0:T, :, 0:HD // 2],
                            in1=qk4[:, 0:T, :, HD // 2:HD],
                            op=mybir.AluOpType.add)
                        t2 = wp.tile([128, SLAB_T, H, HD // 4], BF16, tag="t2")
                        nc.vector.tensor_tensor(
                            out=t2[:, 0:T, :, :],
                            in0=t1[:, 0:T, :, 0:HD // 4],
                            in1=t1[:, 0:T, :, HD // 4:HD // 2],
                            op=mybir.AluOpType.add)
                        t3 = wp.tile([128, SLAB_T, H, HD // 8], BF16, tag="t3")
                        nc.vector.tensor_tensor(
                            out=t3[:, 0:T, :, :],
                            in0=t2[:, 0:T, :, 0:HD // 8],
                            in1=t2[:, 0:T, :, HD // 8:HD // 4],
                            op=mybir.AluOpType.add)
                        sc = wp.tile([128, SLAB_T, H], F32, tag="sc")
                        nc.vector.tensor_reduce(
                            out=sc[:, 0:T, :], in_=t3[:, 0:T, :, :],
                            axis=mybir.AxisListType.X, op=mybir.AluOpType.add)
                        ws = wp.tile([128, SLAB_T, H], F32, tag="ws")
                        nc.gpsimd.tensor_tensor(
                            out=ws[:, 0:T, :], in0=sc[:, 0:T, :],
                            in1=ew_v, op=mybir.AluOpType.mult)
                        ex = wp.tile([128, SLAB_T, H], BF16, tag="ex")
                        nc.scalar.activation(
                            out=ex[:, 0:T, :], in_=ws[:, 0:T, :],
                            func=mybir.ActivationFunctionType.Exp)

                        ctb = wp.tile([128, SLAB_T, D + H], BF16, tag="ctb")
                        nc.vector.tensor_copy(
                            ctb[:, 0:T, D:D + H], ex[:, 0:T, :])
                        nc.gpsimd.tensor_tensor(
                            out=ctb[:, 0:T, 0:D]
                                .rearrange("p t (e h) -> p t e h", h=H),
                            in0=kvg[:, 0:T, D:2 * D]
                                .rearrange("p t (e h) -> p t e h", h=H),
                            in1=ex[:, 0:T, :]
                                .rearrange("p t (o h) -> p t o h", o=1)
                                .to_broadcast([128, T, HD, H]),
                            op=mybir.AluOpType.mult)

                        # segment-sum into per-cell psum slices; flush 4
                        # consecutive blocks per DVE add (amortize PSUM init)
                        for i in range(T):
                            for (blk, lo, hi, first, last, cell) in \
                                    sl["pieces"][i]:
                                grp = blk // 4
                                key = (c, grp)
                                if key not in live:
                                    live[key] = bpp.tile(
                                        [128, 4, D + H], F32, tag="bps",
                                        name=f"bps{c}_{grp}")
                                bps = live[key]
                                nc.tensor.matmul(
                                    out=bps[:, blk % 4, :],
                                    lhsT=sel[lo:hi, :, i],
                                    rhs=ctb[lo:hi, i, :],
                                    start=first, stop=last)
                                if last and blk == min(4 * grp + 3, NBLK - 1):
                                    nb_g = blk - 4 * grp + 1
                                    nc.vector.tensor_tensor(
                                        out=acc[:, 4 * grp:blk + 1, :],
                                        in0=acc[:, 4 * grp:blk + 1, :],
                                        in1=bps[:, 0:nb_g, :],
                                        op=mybir.AluOpType.add)
                                    del live[key]

                    # ---- pipeline: build chunk 0, then A/B/C software
                    # pipeline over slabs with chunk c+1 builds interleaved
                    NGRP = [(r + 2047) // 2048 for r in CHUNK_ROWS]
                    for g in range(NGRP[0]):
                        build_kv_group(0, g)
                    ns = len(slabs)
                    # build-group schedule: spread chunk c+1's groups
                    # over chunk c's A-stage iterations
                    binj = {}
                    for c in range(CH - 1):
                        idxs = [i for i, s in enumerate(slabs)
                                if s["chunk"] == c]
                        ng = NGRP[c + 1]
                        for j, g in enumerate(range(ng)):
                            at = idxs[min(j * len(idxs) // ng,
                                          len(idxs) - 1)]
                            binj.setdefault(at, []).append((c + 1, g))
                    slab_A(slabs[0])
                    for k in range(ns + 2):
                        if k - 2 >= 0:
                            slab_C(slabs[k - 2])
                        if k < ns:
                            for (bc, bg) in binj.get(k, []):
                                build_kv_group(bc, bg)
                        if k + 1 < ns:
                            slab_A(slabs[k + 1])
                        if 0 <= k - 1 < ns:
                            slab_B(slabs[k - 1])

                # ---- epilogue over all blocks
                with (
                    tc.tile_pool(name="ep", bufs=3) as ep,
                    tc.tile_pool(name="ett", bufs=2, space="PSUM") as et,
                    tc.tile_pool(name="eo", bufs=2, space="PSUM") as eo,
                ):
                    EB = 14                       # blocks per epilogue slab
                    for s0 in range(0, NBLK, EB):
                        nb = min(EB, NBLK - s0)
                        zr = ep.tile([128, EB, H], F32, tag="zr")
                        nc.vector.tensor_scalar_add(
                            zr[:, 0:nb, :], acc[:, s0:s0 + nb, D:D + H],
                            1e-16)
                        nc.vector.reciprocal(zr[:, 0:nb, :], zr[:, 0:nb, :])
                        vals = ep.tile([128, EB, D], F32, tag="vals")
                        nc.gpsimd.tensor_tensor(
                            out=vals[:, 0:nb, :]
                                .rearrange("p b (e h) -> p b e h", h=H),
                            in0=acc[:, s0:s0 + nb, 0:D]
                                .rearrange("p b (e h) -> p b e h", h=H),
                            in1=zr[:, 0:nb, :]
                                .rearrange("p b (o h) -> p b o h", o=1)
                                .to_broadcast([128, nb, HD, H]),
                            op=mybir.AluOpType.mult)
                        po = eo.tile([128, EB, D], F32, tag="po")
                        for i in range(nb):
                            pt = et.tile([D, 128], F32, tag="pt")
                            nc.tensor.transpose(
                                out=pt[:], in_=vals[:, i, :],
                                identity=idnf_t[:])
                            vT = ep.tile([D, 128], BF16, tag="vT")
                            nc.scalar.copy(vT[:], pt[:])
                            nc.tensor.matmul(
                                out=po[:, i, :], lhsT=vT[:], rhs=wot_t[:],
                                start=True, stop=True)
                        nmu = ep.tile([128, EB], F32, tag="nmu")
                        nc.vector.tensor_reduce(
                            out=nmu[:, 0:nb], in_=po[:, 0:nb, :],
                            axis=mybir.AxisListType.X, op=mybir.AluOpType.add)
                        nc.vector.tensor_scalar_mul(
                            nmu[:, 0:nb], nmu[:, 0:nb], -1.0 / D)
                        ct = ep.tile([128, EB, D], F32, tag="ct")
                        nc.vector.tensor_tensor(
                            out=ct[:, 0:nb, :], in0=po[:, 0:nb, :],
                            in1=nmu[:, 0:nb].rearrange("p (b o) -> p b o", o=1)
                                .to_broadcast([128, nb, D]),
                            op=mybir.AluOpType.add)
                        nc.gpsimd.tensor_tensor(
                            out=ct[:, 0:nb, :], in0=ct[:, 0:nb, :],
                            in1=boc_t[:].rearrange("p (o d) -> p o d", o=1)
                                .to_broadcast([128, nb, D]),
                            op=mybir.AluOpType.add)
                        sq = ep.tile([128, EB, D], F32, tag="sq")
                        nc.scalar.square(sq[:, 0:nb, :], ct[:, 0:nb, :])
                        v1 = ep.tile([128, EB], F32, tag="v1")
                        nc.vector.tensor_reduce(
                            out=v1[:, 0:nb], in_=sq[:, 0:nb, :],
                            axis=mybir.AxisListType.X, op=mybir.AluOpType.add)
                        nc.vector.tensor_scalar(
                            out=v1[:, 0:nb], in0=v1[:, 0:nb],
                            scalar1=1.0 / D, scalar2=LN_EPS,
                            op0=mybir.AluOpType.mult,
                            op1=mybir.AluOpType.add)
                        nc.vector.reciprocal(v1[:, 0:nb], v1[:, 0:nb])
                        rstd = ep.tile([128, EB], F32, tag="rstd")
                        nc.scalar.sqrt(rstd[:, 0:nb], v1[:, 0:nb])
                        xb = ep.tile([128, EB, D], BF16, tag="xb")
                        nc.sync.dma_start(
                            out=xb[:, 0:nb, :],
                            in_=xpb[s0 * 128:(s0 + nb) * 128, :]
                                .rearrange("(a p) d -> p a d", p=128))
                        ot = ep.tile([128, EB, D], F32, tag="ot")
                        nc.gpsimd.tensor_tensor(
                            out=ot[:, 0:nb, :], in0=ct[:, 0:nb, :],
                            in1=rstd[:, 0:nb]
                                .rearrange("p (b o) -> p b o", o=1)
                                .to_broadcast([128, nb, D]),
                            op=mybir.AluOpType.mult)
                        nc.gpsimd.tensor_tensor(
                            out=ot[:, 0:nb, :], in0=ot[:, 0:nb, :],
                            in1=gam_t[:].rearrange("p (o d) -> p o d", o=1)
                                .to_broadcast([128, nb, D]),
                            op=mybir.AluOpType.mult)
                        nc.gpsimd.tensor_tensor(
                            out=ot[:, 0:nb, :], in0=ot[:, 0:nb, :],
                            in1=xb[:, 0:nb, :], op=mybir.AluOpType.add)
                        nc.sync.dma_start(
                            out=out[s0 * 128:(s0 + nb) * 128, :]
                                .rearrange("(a p) d -> p a d", p=128),
                            in_=ot[:, 0:nb, :])
    return nc


def kernel(x, edge_index, edge_weights, Wq, bq, Wk, bk, Wv, bv, Wo, bo,
           gamma, beta):
    x = np.asarray(x, np.float32)
    edge_index = np.asarray(edge_index)
    edge_weights = np.asarray(edge_weights, np.float32)
    origins = np.asarray(edge_index[0], np.int64)
    dests = np.asarray(edge_index[1], np.int64)

    struct, per_core_raw = _build_structure(origins, dests)
    nc = _build_graph(struct)
    nc.finalize()

    xT = np.zeros((D + 1, NT), np.float32)
    xT[:D, :N] = x.T
    xT[D] = 1.0
    xT = xT.astype(BF16_NP)
    vperm = (np.arange(H)[None, :] * HD + np.arange(HD)[:, None]).ravel()
    wkv = np.zeros((D + 1, 2 * D), np.float32)
    wkv[:D, :D] = np.asarray(Wk, np.float32).T
    wkv[:D, D:] = np.asarray(Wv, np.float32).T[:, vperm]
    wkv[D, :D] = np.asarray(bk, np.float32)
    wkv[D, D:] = np.asarray(bv, np.float32)[vperm]
    wq = np.zeros((D + 1, D), np.float32)
    wq[:D, :] = np.asarray(Wq, np.float32).T
    wq[D, :] = np.asarray(bq, np.float32)
    wkv = wkv.astype(BF16_NP)
    wq = wq.astype(BF16_NP)
    wot = np.ascontiguousarray(np.asarray(Wo, np.float32).T[vperm, :])
    bo_ = np.asarray(bo, np.float32)
    boc = np.tile((bo_ - bo_.mean())[None, :], (128, 1)).astype(np.float32)
    gam_t = np.tile(np.asarray(gamma, np.float32)[None, :], (128, 1))
    iot = np.tile(np.arange(128, dtype=np.float32)[None, :, None],
                  (128, 1, SLAB_T)).astype(BF16_NP)
    idn = np.eye(128, dtype=np.float32)

    in_maps = []
    for c in range(NCORES):
        data = _per_core_arrays(struct, per_core_raw[c], edge_weights)
        xTo = np.zeros((D + 1, NB), np.float32)
        xTo[:D, :NOWN] = x[c * NOWN:(c + 1) * NOWN].T
        xTo[D] = 1.0
        xTo = xTo.astype(BF16_NP)
        xpb = np.zeros((NB, D), np.float32)
        xpb[:NOWN] = x[c * NOWN:(c + 1) * NOWN] + np.asarray(beta, np.float32)
        in_maps.append({
            "xT": xT, "xTo": xTo, "wkv": wkv, "wq": wq, "wot": wot,
            "boc": boc, "gam": gam_t, "iot": iot,
            "idn": idn.astype(BF16_NP), "idnf": idn,
            "xpb": xpb.astype(BF16_NP), "met": data["meta"],
        })

    global LAST_SIM_NS
    if SIMULATE_COST:
        from concourse import bass_interp
        sim = bass_interp.CoreSim(nc, no_exec=True, publish_trace=False)
        sim.event_loop()
        LAST_SIM_NS = int(sim.time)

    res = run_bass_kernel_spmd(nc, in_maps, core_ids=list(range(NCORES)),
                               trace=TRACE)
    global LAST_RESULT
    LAST_RESULT = res
    outs = [np.asarray(res.results[i]["out"])[:NOWN] for i in range(NCORES)]
    return np.concatenate(outs, axis=0).astype(np.float32)


TRACE = False
SIMULATE_COST = False
LAST_RESULT = None
LAST_SIM_NS = None


# revision 8
# speedup vs baseline: 1.4836x; 1.0251x over previous
"""Trainium2 Bass kernel v2 for BaseDependentAttentionLayer (GNN message passing).

Design vs v1 baseline (739us cost-model):
  - Chunk-outer pipeline: kv-table chunk c+1 builds (PE/DMA) while chunk c's
    edges process (DVE/Pool/Act).  Removes the 175us serial phase-1 stall.
  - No q gather: per tile, PE transposes the sel one-hot (slot x origin),
    Act copies selT to SBUF, PE matmul broadcasts the 128-node q block to
    slots.  Kills 250k gather descs of DMA + Pool desc-gen.
  - Per-block accumulators live in SBUF f32 across all 4 chunks; single
    final epilogue (one act-table switch, big-slab instructions).
  - Metadata packed per slab in one i16 stream (kvi | oid | ew-bf16).
  - Engine placement: sel/qk/tree/reduce/ctb on DVE; ws + psum flushes +
    phase1 copies on Pool; selT copies + exp on Act; DMA issues on SP.
"""

import sys

sys.path.insert(0, "/opt/trn_rl_repo")

import numpy as np
import ml_dtypes

import concourse.bass as bass
import concourse.bacc as bacc
import concourse.mybir as mybir
from concourse.tile import TileContext
from concourse.bass_utils import run_bass_kernel_spmd

N = 100000
E = 1600000
D = 64
H = 4
HD = 16
NCORES = 8
NOWN = 12500            # nodes owned per core
NBLK = 98               # 128-node blocks per core
NB = NBLK * 128         # 12544 padded own nodes
NT = 100352             # padded global table rows (= 4 * 25088)
CH = 4                  # dest chunks (int16 gather index limit)
# uneven chunks: tiny chunk 0 builds fast so edge processing starts early;
# the big chunks build in the shadow of processing (all rows <= 32767)
CHUNK_BASE = [0, 16384, 45056, 72704]
CHUNK_ROWS = [16384, 28672, 27648, 27648]
SLAB_T = 48             # edge tiles per processing slab
QB = 8                  # tiles per selT/q-broadcast batch
LN_EPS = 1e-5
PAD_OID = 200.0

F32 = mybir.dt.float32
BF16 = mybir.dt.bfloat16
I16 = mybir.dt.int16
BF16_NP = ml_dtypes.bfloat16


def _build_structure(origins, dests):
    """Chunk-major packed cell layout (no per-cell 128-rounding).

    Cells (chunk, block) are packed back-to-back; only chunk streams are
    padded to 128.  Tiles spanning cell boundaries are handled with
    partition-sliced matmuls ("pieces").  Structure is shared by cores:
    per-cell slot count = max over cores.
    """
    owner = origins // NOWN
    per_core_raw = []
    cnts = np.zeros((NCORES, CH * NBLK), np.int64)
    for c in range(NCORES):
        m = owner == c
        o = (origins[m] - c * NOWN).astype(np.int32)
        d = dests[m].astype(np.int32)
        eids = np.nonzero(m)[0]
        blk = o >> 7
        chunk = np.searchsorted(np.asarray(CHUNK_BASE), d, side="right") - 1
        cell = chunk * NBLK + blk          # chunk-major
        order = np.argsort(cell, kind="stable")
        o, d, eids = o[order], d[order], eids[order]
        cnt = np.bincount(cell[order], minlength=CH * NBLK)
        cnts[c] = cnt
        per_core_raw.append((o, d, eids, cnt))
    # 128-aligned cells: base-64 matmul pieces compile but fail at runtime
    cell_slots = ((cnts.max(0) + 127) // 128) * 128

    cell_off = np.zeros(CH * NBLK, np.int64)
    pieces_by_tile = {}
    tile_total = 0
    chunk_ranges = []                      # (tile0, ntiles) per chunk
    for c in range(CH):
        s0 = tile_total * 128
        pos = s0
        for b in range(NBLK):
            cell = c * NBLK + b
            cell_off[cell] = pos
            n = int(cell_slots[cell])
            if n == 0:
                continue
            p0 = pos
            while p0 < pos + n:
                t = p0 // 128
                lo = p0 - t * 128
                hi = min(128, pos + n - t * 128)
                pieces_by_tile.setdefault(t, []).append(
                    (b, lo, hi, p0 == pos, t * 128 + hi == pos + n, cell))
                p0 = t * 128 + hi
            pos += n
        ntiles = (pos - s0 + 127) // 128
        chunk_ranges.append((tile_total, ntiles))
        tile_total += ntiles
    S_tiles = tile_total

    slabs = []
    meta_off = 0
    for c in range(CH):
        t0c, tc = chunk_ranges[c]
        s = t0c
        while s < t0c + tc:
            T = min(SLAB_T, t0c + tc - s)
            slabs.append({
                "chunk": c, "t0": s, "T": T, "meta_off": meta_off,
                "pieces": {t - s: pieces_by_tile.get(t, [])
                           for t in range(s, s + T)},
            })
            meta_off += 13 * T
            s += T
    struct = {
        "slabs": slabs,
        "S_tiles": S_tiles,
        "meta_cols": meta_off,
        "cell_slots": cell_slots,
        "cell_off": cell_off,
    }
    return struct, per_core_raw


def _per_core_arrays(struct, core_raw, edge_weights):
    """Packed per-slab metadata stream [128, meta_cols] int16."""
    o, d, eids, cnt = core_raw
    S_tiles = struct["S_tiles"]
    S = S_tiles * 128
    cell_off = struct["cell_off"]

    oid = np.full(S, PAD_OID, np.float32)
    kvi = np.zeros(S, np.int16)
    ew4 = np.zeros((S, H), np.float32)

    cell_edge_off = np.zeros(CH * NBLK + 1, np.int64)
    np.cumsum(cnt, out=cell_edge_off[1:])
    for cell in range(CH * NBLK):
        n = int(cnt[cell])
        if n == 0:
            continue
        e0 = cell_edge_off[cell]
        s0 = int(cell_off[cell])
        ch = cell // NBLK
        sl = slice(s0, s0 + n)
        el = slice(e0, e0 + n)
        oid[sl] = (o[el] & 127).astype(np.float32)
        kvi[sl] = (d[el] - CHUNK_BASE[ch]).astype(np.int16)
        ew4[sl] = edge_weights[eids[el]] * (HD ** -0.5)

    # slot-major -> tile layouts
    oid_t = np.ascontiguousarray(oid.reshape(S_tiles, 128).T).astype(BF16_NP)
    ew_t = np.ascontiguousarray(
        ew4.reshape(S_tiles, 128, H).transpose(1, 0, 2)).astype(BF16_NP)

    def wrap(run_vals):
        w = run_vals.reshape(-1, 16).T          # [16, len/16]
        return np.tile(w, (8, 1))               # [128, len/16]

    meta = np.zeros((128, struct["meta_cols"]), np.int16)
    for sl in struct["slabs"]:
        t0, T, mo = sl["t0"], sl["T"], sl["meta_off"]
        meta[:, mo:mo + 8 * T] = wrap(kvi[t0 * 128:(t0 + T) * 128])
        meta[:, mo + 8 * T:mo + 9 * T] = oid_t[:, t0:t0 + T].view(np.int16)
        meta[:, mo + 9 * T:mo + 13 * T] = (
            ew_t[:, t0:t0 + T, :].reshape(128, T * H).view(np.int16))
    return {"meta": meta}


def _build_graph(struct):
    nc = bacc.Bacc()
    slabs = struct["slabs"]

    xT = nc.declare_dram_parameter("xT", [D + 1, NT], BF16, isOutput=False)
    xTo = nc.declare_dram_parameter("xTo", [D + 1, NB], BF16, isOutput=False)
    wkv = nc.declare_dram_parameter("wkv", [D + 1, 2 * D], BF16, isOutput=False)
    wq = nc.declare_dram_parameter("wq", [D + 1, D], BF16, isOutput=False)
    wot = nc.declare_dram_parameter("wot", [D, D], F32, isOutput=False)
    boc = nc.declare_dram_parameter("boc", [128, D], F32, isOutput=False)
    gam = nc.declare_dram_parameter("gam", [128, D], F32, isOutput=False)
    iot = nc.declare_dram_parameter("iot", [128, 128, SLAB_T], BF16,
                                    isOutput=False)
    idn = nc.declare_dram_parameter("idn", [128, 128], BF16, isOutput=False)
    idnf = nc.declare_dram_parameter("idnf", [128, 128], F32, isOutput=False)
    xpb = nc.declare_dram_parameter("xpb", [NB, D], BF16, isOutput=False)
    met = nc.declare_dram_parameter("met", [128, struct["meta_cols"]], I16,
                                    isOutput=False)
    out = nc.declare_dram_parameter("out", [NB, D], F32, isOutput=True)

    kv_tab = nc.dram_tensor("kv_tab", [NT, 2 * D], BF16)

    with TileContext(nc) as tc:
        with tc.tile_pool(name="const", bufs=1) as cp:
            wkv_t = cp.tile([D + 1, 2 * D], BF16)
            nc.sync.dma_start(out=wkv_t[:], in_=wkv[:])
            wq_t = cp.tile([D + 1, D], BF16)
            nc.sync.dma_start(out=wq_t[:], in_=wq[:])
            wot_f = cp.tile([D, D], F32)
            nc.sync.dma_start(out=wot_f[:], in_=wot[:])
            wot_t = cp.tile([D, D], BF16)
            nc.vector.tensor_copy(wot_t[:], wot_f[:])
            boc_t = cp.tile([128, D], F32)
            nc.sync.dma_start(out=boc_t[:], in_=boc[:])
            gam_t = cp.tile([128, D], F32)
            nc.sync.dma_start(out=gam_t[:], in_=gam[:])
            iot_t = cp.tile([128, 128, SLAB_T], BF16)
            nc.sync.dma_start(out=iot_t[:], in_=iot[:])
            idn_t = cp.tile([128, 128], BF16)
            nc.sync.dma_start(out=idn_t[:], in_=idn[:])
            idnf_t = cp.tile([128, 128], F32)
            nc.sync.dma_start(out=idnf_t[:], in_=idnf[:])

            # q blocks for all own nodes, SBUF-resident [128, NBLK, D] bf16
            q_all = cp.tile([128, NBLK, D], BF16)
            # per-block accumulators [128, NBLK, D+H]; bf16 halves SBUF
            # (4 chunk-partials per node round at ~0.3% rel, within margin)
            acc = cp.tile([128, NBLK, D + H], BF16)
            nc.gpsimd.memset(acc[:], 0.0)

            # ---- q blocks: lhsT = xTo block cols, rhs = wq -> [128, 64]
            with (
                tc.tile_pool(name="qbx", bufs=2) as qbx,
                tc.tile_pool(name="qbp", bufs=2, space="PSUM") as qbp,
            ):
                for bat in range((NBLK + 7) // 8):
                    b0 = bat * 8
                    nb = min(8, NBLK - b0)
                    xs = qbx.tile([D + 1, 8 * 128], BF16, tag="xq")
                    nc.scalar.dma_start(
                        out=xs[:, 0:nb * 128],
                        in_=xTo[:, b0 * 128:(b0 + nb) * 128])
                    ps = qbp.tile([128, 8, D], F32, tag="qp")
                    for j in range(nb):
                        nc.tensor.matmul(
                            out=ps[:, j, :],
                            lhsT=xs[:, j * 128:(j + 1) * 128],
                            rhs=wq_t[:],
                            start=True, stop=True)
                    nc.vector.tensor_copy(
                        q_all[:, b0:b0 + nb, :], ps[:, 0:nb, :])

            with (
                tc.tile_pool(name="p1x", bufs=2) as p1x,
                tc.tile_pool(name="p1s", bufs=2) as p1s,
                tc.tile_pool(name="p1p", bufs=2, space="PSUM") as p1p,
            ):
                # ---- kv table chunk builder: groups of 16 blocks (2048 rows)
                def build_kv_group(c, g):
                    # chunk 0 builds serially at startup while Act/SP are
                    # otherwise idle: alternate load/write across queues
                    ld = (nc.gpsimd if g % 2 == 0 else nc.sync) \
                        if c == 0 else nc.sync
                    wr = (nc.sync if g % 2 == 0 else nc.gpsimd) \
                        if c == 0 else nc.sync
                    r0 = CHUNK_BASE[c] + g * 2048
                    nrow = min(2048, CHUNK_ROWS[c] - g * 2048)
                    nblk4 = (nrow + 511) // 512     # 4-block psum batches
                    xs = p1x.tile([D + 1, 2048], BF16, tag="xs")
                    ld.dma_start(
                        out=xs[:, 0:nrow], in_=xT[:, r0:r0 + nrow])
                    sb = p1s.tile([128, 16, 2 * D], BF16, tag="sb")
                    for bat in range(nblk4):
                        ps = p1p.tile([128, 4, 2 * D], F32, tag="kp")
                        for j in range(4):
                            nc.tensor.matmul(
                                out=ps[:, j, :],
                                lhsT=xs[:, bat * 512 + j * 128:
                                        bat * 512 + (j + 1) * 128],
                                rhs=wkv_t[:],
                                start=True, stop=True)
                        if c == 0 and bat % 2 == 0:
                            nc.vector.tensor_copy(
                                sb[:, bat * 4:(bat + 1) * 4, :]
                                    .rearrange("p a d -> p (a d)"),
                                ps[:].rearrange("p a d -> p (a d)"))
                        else:
                            nc.scalar.copy(
                                sb[:, bat * 4:(bat + 1) * 4, :]
                                    .rearrange("p a d -> p (a d)"),
                                ps[:].rearrange("p a d -> p (a d)"))
                    wr.dma_start(
                        out=kv_tab[r0:r0 + nrow, :]
                            .rearrange("(a p) d -> p a d", p=128),
                        in_=sb[:, 0:nrow // 128, :])

                # ---- edge-processing slab
                with (
                    tc.tile_pool(name="gat", bufs=3) as gp,
                    tc.tile_pool(name="mp", bufs=3) as mp,
                    tc.tile_pool(name="sp", bufs=4) as sp,
                    tc.tile_pool(name="qs", bufs=3) as qp,
                    tc.tile_pool(name="wk", bufs=2) as wp,
                    tc.tile_pool(name="stp", bufs=2, space="PSUM") as stp,
                    tc.tile_pool(name="qpp", bufs=2, space="PSUM") as qpp,
                    tc.tile_pool(name="bpp", bufs=2, space="PSUM") as bpp,
                ):
                    state = {}
                    live = {}

                    def slab_A(sl):
                        c, t0, T = sl["chunk"], sl["t0"], sl["T"]
                        mo = sl["meta_off"]
                        mt = mp.tile([128, 13 * SLAB_T], I16, tag="mt")
                        nc.sync.dma_start(
                            out=mt[:, 0:13 * T], in_=met[:, mo:mo + 13 * T])
                        kvi_v = mt[:, 0:8 * T]
                        kvg = gp.tile([128, SLAB_T, 2 * D], BF16, tag="kvg")
                        nc.gpsimd.dma_gather(
                            out_ap=kvg[:, 0:T, :],
                            in_ap=kv_tab[CHUNK_BASE[c]:
                                          CHUNK_BASE[c] + CHUNK_ROWS[c], :],
                            idxs_ap=kvi_v,
                            num_idxs=T * 128,
                            num_idxs_reg=T * 128,
                            elem_size=2 * D,
                            single_packet=False)
                        oid_v = mt[:, 8 * T:9 * T].bitcast(BF16)
                        sel = sp.tile([128, 128, SLAB_T], BF16, tag="sel")
                        nc.vector.tensor_tensor(
                            out=sel[:, :, 0:T],
                            in0=oid_v.rearrange("p (o t) -> p o t", o=1)
                                .to_broadcast([128, 128, T]),
                            in1=iot_t[:, :, 0:T],
                            op=mybir.AluOpType.is_equal)
                        state[sl["t0"]] = (mt, kvg, sel)

                    def slab_B(sl):
                        c, t0, T = sl["chunk"], sl["t0"], sl["T"]
                        mt, kvg, sel = state[t0]

                        # q broadcast: transpose sel -> selT, q = selT^T @ Q,
                        # qk = q*k straight from PSUM (gpsimd can't read PSUM)
                        qk = qp.tile([128, SLAB_T, D], BF16, tag="qk")
                        nbat = (T + QB - 1) // QB
                        for b in range(nbat):
                            i0 = b * QB
                            nb_ = min(QB, T - i0)
                            tp = stp.tile([128, QB, 128], BF16, tag="tp")
                            for j in range(nb_):
                                nc.tensor.transpose(
                                    out=tp[:, j, :],
                                    in_=sel[:, :, i0 + j],
                                    identity=idn_t[:])
                            st = sp.tile([128, QB, 128], BF16, tag="st")
                            nc.scalar.copy(
                                st[:, 0:nb_, :].rearrange("p a d -> p (a d)"),
                                tp[:, 0:nb_, :].rearrange("p a d -> p (a d)"))
                            qps = qpp.tile([128, QB, D], F32, tag="qps")
                            for j in range(nb_):
                                for (blk, lo, hi, _f, _l, _c) in \
                                        sl["pieces"][i0 + j]:
                                    nc.tensor.matmul(
                                        out=qps[lo:hi, j, :],
                                        lhsT=st[:, j, lo:hi],
                                        rhs=q_all[:, blk, :],
                                        start=True, stop=True)
                            nc.vector.tensor_tensor(
                                out=qk[:, i0:i0 + nb_, :],
                                in0=qps[:, 0:nb_, :],
                                in1=kvg[:, i0:i0 + nb_, 0:D],
                                op=mybir.AluOpType.mult)
                        state[t0] = (mt, kvg, sel, qk)

                    def slab_C(sl):
                        c, t0, T = sl["chunk"], sl["t0"], sl["T"]
                        mt, kvg, sel, qk = state.pop(t0)
                        ew_v = mt[:, 9 * T:13 * T].bitcast(BF16) \
                            .rearrange("p (t h) -> p t h", h=H)

                        t1 = wp.tile([128, SLAB_T, H, HD // 2], BF16, tag="t1")
                        qk4 = qk.rearrange("p t (h d) -> p t h d", h=H)
                        nc.gpsimd.tensor_tensor(
                            out=t1[:, 0:T, :, :],
                            in0=qk4[:, 0:T, :, 0:HD // 2],
                            in1=qk4[:, 0:T, :, HD // 2:HD],
                            op=mybir.AluOpType.add)
                        t2 = wp.tile([128, SLAB_T, H, HD // 4], BF16, tag="t2")
                        nc.vector.tensor_tensor(
                            out=t2[:, 0:T, :, :],
                            in0=t1[:, 0:T, :, 0:HD // 4],
                            in1=t1[:, 0:T, :, HD // 4:HD // 2],
                            op=mybir.AluOpType.add)
                        t3 = wp.tile([128, SLAB_T, H, HD // 8], BF16, tag="t3")
                        nc.vector.tensor_tensor(
                            out=t3[:, 0:T, :, :],
                            in0=t2[:, 0:T, :, 0:HD // 8],
                            in1=t2[:, 0:T, :, HD // 8:HD // 4],
                            op=mybir.AluOpType.add)
                        sc = wp.tile([128, SLAB_T, H], F32, tag="sc")
                        nc.vector.tensor_tensor(
                            out=sc[:, 0:T, :],
                            in0=t3[:, 0:T, :, 0],
                            in1=t3[:, 0:T, :, 1],
                            op=mybir.AluOpType.add)
                        ws = wp.tile([128, SLAB_T, H], F32, tag="ws")
                        nc.gpsimd.tensor_tensor(
                            out=ws[:, 0:T, :], in0=sc[:, 0:T, :],
                            in1=ew_v, op=mybir.AluOpType.mult)
                        ex = wp.tile([128, SLAB_T, H], BF16, tag="ex")
                        nc.scalar.activation(
                            out=ex[:, 0:T, :], in_=ws[:, 0:T, :],
                            func=mybir.ActivationFunctionType.Exp)

                        ctb = wp.tile([128, SLAB_T, D + H], BF16, tag="ctb")
                        nc.vector.tensor_copy(
                            ctb[:, 0:T, D:D + H], ex[:, 0:T, :])
                        nc.gpsimd.tensor_tensor(
                            out=ctb[:, 0:T, 0:D]
                                .rearrange("p t (e h) -> p t e h", h=H),
                            in0=kvg[:, 0:T, D:2 * D]
                                .rearrange("p t (e h) -> p t e h", h=H),
                            in1=ex[:, 0:T, :]
                                .rearrange("p t (o h) -> p t o h", o=1)
                                .to_broadcast([128, T, HD, H]),
                            op=mybir.AluOpType.mult)

                        # segment-sum into per-cell psum slices; flush 4
                        # consecutive blocks per DVE add (amortize PSUM init)
                        for i in range(T):
                            for (blk, lo, hi, first, last, cell) in \
                                    sl["pieces"][i]:
                                grp = blk // 4
                                key = (c, grp)
                                if key not in live:
                                    live[key] = bpp.tile(
                                        [128, 4, D + H], F32, tag="bps",
                                        name=f"bps{c}_{grp}")
                                bps = live[key]
                                nc.tensor.matmul(
                                    out=bps[:, blk % 4, :],
                                    lhsT=sel[lo:hi, :, i],
                                    rhs=ctb[lo:hi, i, :],
                                    start=first, stop=last)
                                if last and blk == min(4 * grp + 3, NBLK - 1):
                                    nb_g = blk - 4 * grp + 1
                                    nc.vector.tensor_tensor(
                                        out=acc[:, 4 * grp:blk + 1, :],
                                        in0=acc[:, 4 * grp:blk + 1, :],
                                        in1=bps[:, 0:nb_g, :],
                                        op=mybir.AluOpType.add)
                                    del live[key]

                    # ---- pipeline: build chunk 0, then A/B/C software
                    # pipeline over slabs with chunk c+1 builds interleaved
                    NGRP = [(r + 2047) // 2048 for r in CHUNK_ROWS]
                    for g in range(NGRP[0]):
                        build_kv_group(0, g)
                    ns = len(slabs)
                    # build-group schedule: spread chunk c+1's groups
                    # over chunk c's A-stage iterations
                    binj = {}
                    for c in range(CH - 1):
                        idxs = [i for i, s in enumerate(slabs)
                                if s["chunk"] == c]
                        ng = NGRP[c + 1]
                        for j, g in enumerate(range(ng)):
                            at = idxs[min(j * len(idxs) // ng,
                                          len(idxs) - 1)]
                            binj.setdefault(at, []).append((c + 1, g))
                    slab_A(slabs[0])
                    for k in range(ns + 2):
                        if k - 2 >= 0:
                            slab_C(slabs[k - 2])
                        if k < ns:
                            for (bc, bg) in binj.get(k, []):
                                build_kv_group(bc, bg)
                        if k + 1 < ns:
                            slab_A(slabs[k + 1])
                        if 0 <= k - 1 < ns:
                            slab_B(slabs[k - 1])

                # ---- epilogue over all blocks
                with (
                    tc.tile_pool(name="ep", bufs=3) as ep,
                    tc.tile_pool(name="ett", bufs=2, space="PSUM") as et,
                    tc.tile_pool(name="eo", bufs=2, space="PSUM") as eo,
                ):
                    EB = 10                       # blocks per epilogue slab
                    for s0 in range(0, NBLK, EB):
                        nb = min(EB, NBLK - s0)
                        zr = ep.tile([128, EB, H], F32, tag="zr")
                        nc.vector.tensor_scalar_add(
                            zr[:, 0:nb, :], acc[:, s0:s0 + nb, D:D + H],
                            1e-16)
                        nc.vector.reciprocal(zr[:, 0:nb, :], zr[:, 0:nb, :])
                        vals = ep.tile([128, EB, D], F32, tag="vals")
                        nc.gpsimd.tensor_tensor(
                            out=vals[:, 0:nb, :]
                                .rearrange("p b (e h) -> p b e h", h=H),
                            in0=acc[:, s0:s0 + nb, 0:D]
                                .rearrange("p b (e h) -> p b e h", h=H),
                            in1=zr[:, 0:nb, :]
                                .rearrange("p b (o h) -> p b o h", o=1)
                                .to_broadcast([128, nb, HD, H]),
                            op=mybir.AluOpType.mult)
                        po = eo.tile([128, EB, D], F32, tag="po")
                        for i in range(nb):
                            pt = et.tile([D, 128], F32, tag="pt")
                            nc.tensor.transpose(
                                out=pt[:], in_=vals[:, i, :],
                                identity=idnf_t[:])
                            vT = ep.tile([D, 128], BF16, tag="vT")
                            nc.scalar.copy(vT[:], pt[:])
                            nc.tensor.matmul(
                                out=po[:, i, :], lhsT=vT[:], rhs=wot_t[:],
                                start=True, stop=True)
                        nmu = ep.tile([128, EB], F32, tag="nmu")
                        nc.vector.tensor_reduce(
                            out=nmu[:, 0:nb], in_=po[:, 0:nb, :],
                            axis=mybir.AxisListType.X, op=mybir.AluOpType.add)
                        nc.vector.tensor_scalar_mul(
                            nmu[:, 0:nb], nmu[:, 0:nb], -1.0 / D)
                        ct = ep.tile([128, EB, D], F32, tag="ct")
                        nc.vector.tensor_tensor(
                            out=ct[:, 0:nb, :], in0=po[:, 0:nb, :],
                            in1=nmu[:, 0:nb].rearrange("p (b o) -> p b o", o=1)
                                .to_broadcast([128, nb, D]),
                            op=mybir.AluOpType.add)
                        nc.gpsimd.tensor_tensor(
                            out=ct[:, 0:nb, :], in0=ct[:, 0:nb, :],
                            in1=boc_t[:].rearrange("p (o d) -> p o d", o=1)
                                .to_broadcast([128, nb, D]),
                            op=mybir.AluOpType.add)
                        sq = ep.tile([128, EB, D], F32, tag="sq")
                        nc.scalar.square(sq[:, 0:nb, :], ct[:, 0:nb, :])
                        v1 = ep.tile([128, EB], F32, tag="v1")
                        nc.vector.tensor_reduce(
                            out=v1[:, 0:nb], in_=sq[:, 0:nb, :],
                            axis=mybir.AxisListType.X, op=mybir.AluOpType.add)
                        nc.vector.tensor_scalar(
                            out=v1[:, 0:nb], in0=v1[:, 0:nb],
                            scalar1=1.0 / D, scalar2=LN_EPS,
                            op0=mybir.AluOpType.mult,
                            op1=mybir.AluOpType.add)
                        nc.vector.reciprocal(v1[:, 0:nb], v1[:, 0:nb])
                        rstd = ep.tile([128, EB], F32, tag="rstd")
                        nc.scalar.sqrt(rstd[:, 0:nb], v1[:, 0:nb])
                        xb = ep.tile([128, EB, D], BF16, tag="xb")
                        nc.sync.dma_start(
                            out=xb[:, 0:nb, :],
                            in_=xpb[s0 * 128:(s0 + nb) * 128, :]
                                .rearrange("(a p) d -> p a d", p=128))
                        ot = ep.tile([128, EB, D], F32, tag="ot")
                        nc.gpsimd.tensor_tensor(
                            out=ot[:, 0:nb, :], in0=ct[:, 0:nb, :],
                            in1=rstd[:, 0:nb]
                                .rearrange("p (b o) -> p b o", o=1)
                                .to_broadcast([128, nb, D]),
                            op=mybir.AluOpType.mult)
                        nc.gpsimd.tensor_tensor(
                            out=ot[:, 0:nb, :], in0=ot[:, 0:nb, :],
                            in1=gam_t[:].rearrange("p (o d) -> p o d", o=1)
                                .to_broadcast([128, nb, D]),
                            op=mybir.AluOpType.mult)
                        nc.gpsimd.tensor_tensor(
                            out=ot[:, 0:nb, :], in0=ot[:, 0:nb, :],
                            in1=xb[:, 0:nb, :], op=mybir.AluOpType.add)
                        nc.sync.dma_start(
                            out=out[s0 * 128:(s0 + nb) * 128, :]
                                .rearrange("(a p) d -> p a d", p=128),
                            in_=ot[:, 0:nb, :])
    return nc


def kernel(x, edge_index, edge_weights, Wq, bq, Wk, bk, Wv, bv, Wo, bo,
           gamma, beta):
    x = np.asarray(x, np.float32)
    edge_index = np.asarray(edge_index)
    edge_weights = np.asarray(edge_weights, np.float32)
    origins = np.asarray(edge_index[0], np.int64)
    dests = np.asarray(edge_index[1], np.int64)

    struct, per_core_raw = _build_structure(origins, dests)
    nc = _build_graph(struct)
    nc.finalize()

    xT = np.zeros((D + 1, NT), np.float32)
    xT[:D, :N] = x.T
    xT[D] = 1.0
    xT = xT.astype(BF16_NP)
    vperm = (np.arange(H)[None, :] * HD + np.arange(HD)[:, None]).ravel()
    wkv = np.zeros((D + 1, 2 * D), np.float32)
    wkv[:D, :D] = np.asarray(Wk, np.float32).T
    wkv[:D, D:] = np.asarray(Wv, np.float32).T[:, vperm]
    wkv[D, :D] = np.asarray(bk, np.float32)
    wkv[D, D:] = np.asarray(bv, np.float32)[vperm]
    wq = np.zeros((D + 1, D), np.float32)
    wq[:D, :] = np.asarray(Wq, np.float32).T
    wq[D, :] = np.asarray(bq, np.float32)
    wkv = wkv.astype(BF16_NP)
    wq = wq.astype(BF16_NP)
    wot = np.ascontiguousarray(np.asarray(Wo, np.float32).T[vperm, :])
    bo_ = np.asarray(bo, np.float32)
    boc = np.tile((bo_ - bo_.mean())[None, :], (128, 1)).astype(np.float32)
    gam_t = np.tile(np.asarray(gamma, np.float32)[None, :], (128, 1))
    iot = np.tile(np.arange(128, dtype=np.float32)[None, :, None],
                  (128, 1, SLAB_T)).astype(BF16_NP)
    idn = np.eye(128, dtype=np.float32)

    in_maps = []
    for c in range(NCORES):
        data = _per_core_arrays(struct, per_core_raw[c], edge_weights)
        xTo = np.zeros((D + 1, NB), np.float32)
        xTo[:D, :NOWN] = x[c * NOWN:(c + 1) * NOWN].T
        xTo[D] = 1.0
        xTo = xTo.astype(BF16_NP)
        xpb = np.zeros((NB, D), np.float32)
        xpb[:NOWN] = x[c * NOWN:(c + 1) * NOWN] + np.asarray(beta, np.float32)
        in_maps.append({
            "xT": xT, "xTo": xTo, "wkv": wkv, "wq": wq, "wot": wot,
            "boc": boc, "gam": gam_t, "iot": iot,
            "idn": idn.astype(BF16_NP), "idnf": idn,
            "xpb": xpb.astype(BF16_NP), "met": data["meta"],
        })

    global LAST_SIM_NS
    if SIMULATE_COST:
        from concourse import bass_interp
        sim = bass_interp.CoreSim(nc, no_exec=True, publish_trace=False)
        sim.event_loop()
        LAST_SIM_NS = int(sim.time)

    res = run_bass_kernel_spmd(nc, in_maps, core_ids=list(range(NCORES)),
                               trace=TRACE)
    global LAST_RESULT
    LAST_RESULT = res
    outs = [np.asarray(res.results[i]["out"])[:NOWN] for i in range(NCORES)]
    return np.concatenate(outs, axis=0).astype(np.float32)


TRACE = False
SIMULATE_COST = False
LAST_RESULT = None
LAST_SIM_NS = None
